# revision 1
# baseline (speedup 1.0000x reference)
"""Matformer GNN message-passing kernel for 8 Trainium2 NeuronCores.

Sharding: nodes in contiguous ranges of 1280 per core (batch is sorted so
this approximates graph sharding); edges sharded by dst node range and
grouped by 128-node chunk. Edge message compute (the dominant cost) is
fully sharded; node-level projections are replicated (cheap); nf is
all-gathered between layers; pooled sums are all-reduced at the end.
"""
import numpy as np

# ---- problem constants (hardcoded per contest rules) ----
N, E, G = 10000, 100000, 256
H, C = 4, 128
NL = 5
NCORES = 8
P = 128
RANGE = 1280                 # nodes per core
NPAD = RANGE * NCORES        # 10240
NT = NPAD // P               # 80 node tiles
CHUNKS = RANGE // P          # 10 chunks per core
ECAP_CHUNK = 1536            # max edges per 128-node chunk
TPC = ECAP_CHUNK // P        # 12 edge tiles per chunk
ET = CHUNKS * TPC            # 120 edge tiles per core
ECAP = ET * P                # 15360 edge slots per core
PTILES = CHUNKS              # pooling node tiles per core (local range)
D4 = H * C                   # 512
D3 = 3 * C                   # 384
GAMMA = 1.0 / (8.0 / 127.0)
INV_SQRT = 1.0 / np.sqrt(3.0 * C)
EPS = 1e-5
BN_S = 1.0 / np.sqrt(1.0 + 1e-5)

_NL_BUILD = NL  # overridable for compile-time experiments


def _prep(inp):
    """Host-side data movement: shard + sort edges, fold constants."""
    f32 = np.float32
    x = np.asarray(inp["x"], f32)
    edge_attr = np.asarray(inp["edge_attr"], f32)
    edge_index = np.asarray(inp["edge_index"]).astype(np.int64)
    batch = np.asarray(inp["batch"]).astype(np.int64)
    src, dst = edge_index[0], edge_index[1]

    host = {}
    # ---- weights (shared across cores) ----
    host["x_fm"] = np.zeros((92, NPAD), f32)
    host["x_fm"][:, :N] = x.T
    host["emb_W"] = np.asarray(inp["emb_W"], f32)
    host["emb_b_r"] = np.asarray(inp["emb_b"], f32).reshape(1, 128)
    host["rbf_W1"] = np.asarray(inp["rbf_W1"], f32)
    host["rbf_W2"] = np.asarray(inp["rbf_W2"], f32)
    host["b1_col"] = np.asarray(inp["rbf_b1"], f32).reshape(128, 1)
    host["b2_col"] = np.asarray(inp["rbf_b2"], f32).reshape(128, 1)
    host["centers_r"] = np.linspace(0.0, 8.0, 128, dtype=f32).reshape(1, 128)

    Wq = np.asarray(inp["Wq"], f32) * INV_SQRT
    bq = np.asarray(inp["bq"], f32) * INV_SQRT
    host["Wq_f"] = Wq
    host["bq_r"] = bq.reshape(NL, 1, D4)
    host["Wk"] = np.asarray(inp["Wk"], f32)
    host["bk_r"] = np.asarray(inp["bk"], f32).reshape(NL, 1, D4)
    host["Wv"] = np.asarray(inp["Wv"], f32)
    host["bv_r"] = np.asarray(inp["bv"], f32).reshape(NL, 1, D4)
    host["We"] = np.asarray(inp["We"], f32)
    host["be_r"] = np.asarray(inp["be"], f32).reshape(NL, 1, D4)
    # be in feature-major per-head-chunk layout [128, 4] per layer
    host["be_fm"] = np.ascontiguousarray(
        np.asarray(inp["be"], f32).reshape(NL, H, C).transpose(0, 2, 1))
    host["Wmu"] = np.asarray(inp["Wmu"], f32)
    host["bmu_r"] = np.asarray(inp["bmu"], f32).reshape(NL, 1, D3)
    host["Wmsg"] = np.asarray(inp["Wmsg"], f32)
    host["bmsg_r"] = np.asarray(inp["bmsg"], f32).reshape(NL, 1, C)

    ln1_g = np.asarray(inp["ln1_g"], f32)
    ln1_b = np.asarray(inp["ln1_b"], f32)
    ln2_g = np.asarray(inp["ln2_g"], f32)
    ln2_b = np.asarray(inp["ln2_b"], f32)
    host["ln1_trivial"] = bool(
        np.allclose(ln1_g, 1.0) and np.allclose(ln1_b, 0.0))
    host["g1_r"] = np.tile(ln1_g, (1, H)).reshape(NL, 1, H * D3)
    host["b1_r"] = np.tile(ln1_b, (1, H)).reshape(NL, 1, H * D3)

    Wc = np.asarray(inp["Wc"], f32)
    bc = np.asarray(inp["bc"], f32)
    bn_g = np.asarray(inp["bn_g"], f32)
    bn_b = np.asarray(inp["bn_b"], f32)
    colscale = BN_S * bn_g                       # [NL,128]
    rowscale = np.tile(ln2_g, (1, H))            # [NL,512]
    host["Wc_f"] = Wc * rowscale[:, :, None] * colscale[:, None, :]
    host["bc_r"] = (bc * colscale + bn_b).reshape(NL, 1, 128)
    host["ln2b_trivial"] = bool(np.allclose(ln2_b, 0.0))
    # deg-dependent correction for nonzero ln2 beta:
    # out += deg[n] * ((tile(ln2_b,H) @ Wc) * colscale)
    host["wdeg_r"] = np.einsum(
        "lk,lkj->lj", np.tile(ln2_b, (1, H)), Wc) * colscale
    host["wdeg_r"] = host["wdeg_r"].reshape(NL, 1, 128).astype(f32)

    host["fc_W"] = np.asarray(inp["fc_W"], f32)
    host["fc_b_r"] = np.asarray(inp["fc_b"], f32).reshape(1, 128)
    host["out_W"] = np.asarray(inp["out_W"], f32)
    host["out_b_val"] = float(np.asarray(inp["out_b"], f32).reshape(-1)[0])

    # ---- per-core edge sharding ----
    deg = np.bincount(dst, minlength=NPAD).astype(f32)
    percore = []
    for i in range(NCORES):
        lo, hi = RANGE * i, RANGE * (i + 1)
        src_T = np.zeros((P, ET), np.int32)
        doff_T = np.full((P, ET), 999.0, f32)
        ea_s = np.zeros((ECAP, 3), f32)
        for c in range(CHUNKS):
            nlo = lo + c * P
            sel = np.nonzero((dst >= nlo) & (dst < min(nlo + P, N)))[0]
            cnt = len(sel)
            if cnt > ECAP_CHUNK:
                raise ValueError(
                    f"chunk overflow: core {i} chunk {c} has {cnt} edges "
                    f"> {ECAP_CHUNK}")
            base = c * ECAP_CHUNK
            flat_t = (base + np.arange(cnt)) // P
            flat_p = (base + np.arange(cnt)) % P
            src_T[flat_p, flat_t] = src[sel]
            doff_T[flat_p, flat_t] = (dst[sel] - nlo).astype(f32)
            ea_s[base:base + cnt] = edge_attr[sel]
        chunk_idx_T = (lo + np.arange(RANGE).reshape(CHUNKS, P).T
                       ).astype(np.int32)            # [P, CHUNKS]
        gid = np.full(RANGE, 999.0, f32)
        nreal = min(hi, N) - lo
        if nreal > 0:
            gid[:nreal] = batch[lo:lo + nreal].astype(f32)
        gidA_T = gid.reshape(CHUNKS, P).T.copy()
        gidB_T = (gid - 128.0).reshape(CHUNKS, P).T.copy()
        deg_row = np.zeros((1, RANGE), f32)
        deg_row[0, :nreal] = deg[lo:lo + nreal]
        percore.append(dict(src_T=src_T, doff_T=doff_T, ea_s=ea_s,
                            chunk_idx_T=chunk_idx_T, gidA_T=gidA_T,
                            gidB_T=gidB_T, deg_row=deg_row))
    host["percore"] = percore
    return host


def _build(host, nl_build=NL):
    import concourse.bacc as bacc
    import concourse.tile as tile
    from concourse import bass, mybir
    from concourse.masks import make_identity

    f32 = mybir.dt.float32
    i32 = mybir.dt.int32
    AF = mybir.ActivationFunctionType
    OP = mybir.AluOpType
    AX = mybir.AxisListType

    nc = bacc.Bacc("TRN2", target_bir_lowering=False, debug=False,
                   enable_asserts=False, num_devices=NCORES)

    def din(name, shape, dt=f32):
        return nc.dram_tensor(name, list(shape), dt, kind="ExternalInput")

    # weights
    x_fm = din("x_fm", (92, NPAD))
    emb_W = din("emb_W", (92, 128))
    emb_b_r = din("emb_b_r", (1, 128))
    rbf_W1 = din("rbf_W1", (128, 128))
    rbf_W2 = din("rbf_W2", (128, 128))
    b1_col = din("b1_col", (128, 1))
    b2_col = din("b2_col", (128, 1))
    centers_r = din("centers_r", (1, 128))
    Wq_f = din("Wq_f", (NL, 128, D4))
    bq_r = din("bq_r", (NL, 1, D4))
    Wk = din("Wk", (NL, 128, D4))
    bk_r = din("bk_r", (NL, 1, D4))
    Wv = din("Wv", (NL, 128, D4))
    bv_r = din("bv_r", (NL, 1, D4))
    We = din("We", (NL, 128, D4))
    be_r = din("be_r", (NL, 1, D4))
    be_fm = din("be_fm", (NL, 128, H))
    Wmu = din("Wmu", (NL, D3, D3))
    bmu_r = din("bmu_r", (NL, 1, D3))
    Wmsg = din("Wmsg", (NL, D3, C))
    bmsg_r = din("bmsg_r", (NL, 1, C))
    Wc_f = din("Wc_f", (NL, D4, 128))
    bc_r = din("bc_r", (NL, 1, 128))
    fc_W = din("fc_W", (128, 128))
    fc_b_r = din("fc_b_r", (1, 128))
    out_W = din("out_W", (128, 1))
    if not host["ln1_trivial"]:
        g1_r = din("g1_r", (NL, 1, H * D3))
        b1_r = din("b1_r", (NL, 1, H * D3))
    if not host["ln2b_trivial"]:
        wdeg_r = din("wdeg_r", (NL, 1, 128))
        deg_row = din("deg_row", (1, RANGE))
    # per-core data
    src_T = din("src_T", (P, ET), i32)
    doff_T = din("doff_T", (P, ET))
    ea_s = din("ea_s", (ECAP, 3))
    chunk_idx_T = din("chunk_idx_T", (P, CHUNKS), i32)
    gidA_T = din("gidA_T", (P, CHUNKS))
    gidB_T = din("gidB_T", (P, CHUNKS))

    y = nc.dram_tensor("y", [G, 1], f32, kind="ExternalOutput")

    with tile.TileContext(nc) as tc:
        with tc.tile_pool(name="const", bufs=1) as cpool, \
             tc.tile_pool(name="dram", bufs=1, space="DRAM") as dram, \
             tc.tile_pool(name="wts", bufs=1) as wts, \
             tc.tile_pool(name="sb", bufs=3) as sb, \
             tc.tile_pool(name="sb2", bufs=2) as sb2, \
             tc.tile_pool(name="sbc", bufs=2) as sbc, \
             tc.tile_pool(name="ps", bufs=4, space="PSUM") as ps, \
             tc.tile_pool(name="pst", bufs=2, space="PSUM") as pst, \
             tc.tile_pool(name="agg", bufs=2, space="PSUM") as aggp:

            # ---------------- constants ----------------
            ident = cpool.tile([P, P], f32, name="t0")
            make_identity(nc, ident[:])
            iota_i = cpool.tile([P, P], i32, name="t1")
            nc.gpsimd.iota(iota_i[:], pattern=[[1, P]], base=0,
                           channel_multiplier=0)
            iota_f = cpool.tile([P, P], f32, name="t2")
            nc.vector.tensor_copy(iota_f[:], iota_i[:])
            eps_t = cpool.tile([P, 1], f32, name="t3")
            nc.gpsimd.memset(eps_t[:], EPS)
            ones_r = cpool.tile([1, D4], f32, name="t4")
            nc.gpsimd.memset(ones_r[:], 1.0)
            centers_b = cpool.tile([P, P], f32, name="t5")
            nc.sync.dma_start(centers_b[:], centers_r[:].to_broadcast([P, P]))

            _rs_uid = [0]

            def rsqrt(x_ap, shape, eng, iters=2):
                """rsqrt(x) via quake bit-trick + Newton, all on `eng`
                (no ACT table needed). x must be > 0."""
                u = _rs_uid[0]
                _rs_uid[0] += 1
                ish = sb.tile(shape, i32, tag="rs_sh", name=f"rsh{u}")
                eng.tensor_scalar(out=ish[:], in0=x_ap.bitcast(i32),
                                  scalar1=1, scalar2=None,
                                  op0=OP.logical_shift_right)
                y0 = sb.tile(shape, i32, tag="rs_y0", name=f"rsy{u}")
                eng.tensor_scalar(out=y0[:], in0=ish[:], scalar1=-1,
                                  scalar2=0x5f3759df, op0=OP.mult,
                                  op1=OP.add)
                hv = sb.tile(shape, f32, tag="rs_hv", name=f"rsh2{u}")
                eng.tensor_scalar(out=hv[:], in0=x_ap, scalar1=-0.5,
                                  scalar2=None, op0=OP.mult)
                y = y0[:].bitcast(f32)
                for it in range(iters):
                    y2 = sb.tile(shape, f32, tag=f"rs_a{it}",
                                 name=f"rsa{u}_{it}")
                    eng.tensor_tensor(out=y2[:], in0=y, in1=y, op=OP.mult)
                    t2 = sb.tile(shape, f32, tag=f"rs_b{it}",
                                 name=f"rsb{u}_{it}")
                    eng.tensor_tensor(out=t2[:], in0=y2[:], in1=hv[:],
                                      op=OP.mult)
                    yn = sb.tile(shape, f32, tag=f"rs_c{it}",
                                 name=f"rsc{u}_{it}")
                    eng.scalar_tensor_tensor(out=yn[:], in0=t2[:],
                                             scalar=1.5, in1=y,
                                             op0=OP.add, op1=OP.mult)
                    y = yn[:]
                return y

            # persistent per-core edge metadata
            src_sb = cpool.tile([P, ET], i32, name="t6")
            nc.sync.dma_start(src_sb[:], src_T[:])
            doff_sb = cpool.tile([P, ET], f32, name="t7")
            nc.sync.dma_start(doff_sb[:], doff_T[:])
            cidx_sb = cpool.tile([P, CHUNKS], i32, name="t8")
            nc.sync.dma_start(cidx_sb[:], chunk_idx_T[:])

            # ---------------- DRAM scratch ----------------
            nf0 = dram.tile([NPAD, 128], f32, name="t9")
            q_tab = dram.tile([NPAD, D4], f32, name="t10")
            k_tab = dram.tile([NPAD, D4], f32, name="t11")
            v_tab = dram.tile([NPAD, D4], f32, name="t12")
            qk_tab = dram.tile([NPAD, D4], f32, name="t13")
            ef_dram = dram.tile([128, ECAP], f32, name="t14")
            ag_in = [dram.tile([RANGE, 128], f32, name="t15") for _ in range(nl_build)]
            ag_out = [dram.tile([NPAD, 128], f32, addr_space="Shared", name="t16")
                      for _ in range(nl_build)]
            ar_in = dram.tile([2 * P, 129], f32, name="t17")
            ar_out = dram.tile([2 * P, 129], f32, addr_space="Shared", name="t18")

            # ---------------- stage A: nf0 = x @ emb_W + emb_b ------------
            embW_sb = cpool.tile([92, 128], f32, name="t19")
            nc.sync.dma_start(embW_sb[:], emb_W[:])
            embb_sb = cpool.tile([1, 128], f32, name="t20")
            nc.sync.dma_start(embb_sb[:], emb_b_r[:])
            for t in range(NT):
                xt = sb.tile([92, P], f32, tag="xt", name="xt")
                nc.sync.dma_start(xt[:], x_fm[:, t * P:(t + 1) * P])
                pnf = ps.tile([P, 128], f32, tag="w", name="w")
                nc.tensor.matmul(pnf[:], lhsT=xt[:], rhs=embW_sb[:],
                                 start=True, stop=False, skip_group_check=True)
                nc.tensor.matmul(pnf[:], lhsT=ones_r[:, :P], rhs=embb_sb[:],
                                 start=False, stop=True, skip_group_check=True)
                nft = sb.tile([P, 128], f32, tag="nft", name="nft")
                nc.vector.tensor_copy(nft[:], pnf[:])
                nc.sync.dma_start(nf0[t * P:(t + 1) * P, :], nft[:])

            # ---------------- stage B: ef (feature-major) -----------------
            W1_sb = cpool.tile([128, 128], f32, name="t24")
            nc.sync.dma_start(W1_sb[:], rbf_W1[:])
            W2_sb = cpool.tile([128, 128], f32, name="t25")
            nc.sync.dma_start(W2_sb[:], rbf_W2[:])
            b1_sb = cpool.tile([128, 1], f32, name="t26")
            nc.sync.dma_start(b1_sb[:], b1_col[:])
            b2_sb = cpool.tile([128, 1], f32, name="t27")
            nc.sync.dma_start(b2_sb[:], b2_col[:])
            for t in range(ET):
                ea_t = sb.tile([P, 3], f32, tag="ea", name="ea")
                nc.sync.dma_start(ea_t[:], ea_s[t * P:(t + 1) * P, :])
                ssq = sb.tile([P, 1], f32, tag="ssq", name="ssq")
                sq3 = sb.tile([P, 3], f32, tag="sq3", name="sq3")
                nc.scalar.activation(sq3[:], ea_t[:], AF.Square,
                                     accum_out=ssq[:, :1])
                ssq_e = sb.tile([P, 1], f32, tag="ssq_e", name="ssq_e")
                nc.vector.tensor_scalar(out=ssq_e[:], in0=ssq[:],
                                        scalar1=1e-30, scalar2=None,
                                        op0=OP.add)
                rsq = rsqrt(ssq_e[:], [P, 1], nc.vector)
                dlen = sb.tile([P, 1], f32, tag="dlen", name="dlen")
                nc.vector.tensor_tensor(out=dlen[:], in0=ssq[:], in1=rsq,
                                        op=OP.mult)
                diff = sb.tile([P, P], f32, tag="diff", name="diff")
                nc.vector.tensor_scalar(out=diff[:], in0=centers_b[:],
                                        scalar1=dlen[:, :1], scalar2=None,
                                        op0=OP.subtract)
                dsq = sb.tile([P, P], f32, tag="dsq", name="dsq")
                nc.scalar.square(dsq[:], diff[:])
                rbf = sb.tile([P, P], f32, tag="rbf", name="rbf")
                nc.scalar.activation(rbf[:], dsq[:], AF.Exp, scale=-GAMMA)
                prT = pst.tile([P, P], f32, tag="t", name="t")
                nc.tensor.transpose(prT[:], rbf[:], ident[:])
                rbfT = sb.tile([P, P], f32, tag="rbfT", name="rbfT")
                nc.vector.tensor_copy(rbfT[:], prT[:])
                ph = ps.tile([P, P], f32, tag="w", name="w")
                nc.tensor.matmul(ph[:], lhsT=W1_sb[:], rhs=rbfT[:],
                                 start=True, stop=True, skip_group_check=True)
                # softplus = ln(1 + exp(x)) — Exp and Ln live in the same
                # ACT table (natural_log_exp_and_others)
                eh = sb.tile([P, P], f32, tag="eh", name="eh")
                nc.scalar.activation(eh[:], ph[:], AF.Exp,
                                     bias=b1_sb[:, :1])
                hfm = sb.tile([P, P], f32, tag="hfm", name="hfm")
                nc.scalar.activation(hfm[:], eh[:], AF.Ln, bias=1.0)
                pef = ps.tile([P, P], f32, tag="w", name="w")
                nc.tensor.matmul(pef[:], lhsT=W2_sb[:], rhs=hfm[:],
                                 start=True, stop=True, skip_group_check=True)
                eft = sb.tile([P, P], f32, tag="eft", name="eft")
                nc.scalar.activation(eft[:], pef[:], AF.Identity,
                                     bias=b2_sb[:, :1])
                nc.sync.dma_start(ef_dram[:, t * P:(t + 1) * P], eft[:])

            # ---------------- layers ----------------
            for l in range(nl_build):
                nf_src = nf0 if l == 0 else ag_out[l - 1]

                # -- C1: q/k/v/qk tables (all nodes, replicated) --
                Wq_sb = wts.tile([128, D4], f32, tag="Wq", name="Wq")
                nc.sync.dma_start(Wq_sb[:], Wq_f[l])
                Wk_sb = wts.tile([128, D4], f32, tag="Wk", name="Wk")
                nc.sync.dma_start(Wk_sb[:], Wk[l])
                Wv_sb = wts.tile([128, D4], f32, tag="Wv", name="Wv")
                nc.sync.dma_start(Wv_sb[:], Wv[l])
                We_sb = wts.tile([128, D4], f32, tag="We", name="We")
                nc.sync.dma_start(We_sb[:], We[l])
                bq_sb = wts.tile([1, D4], f32, tag="bq", name="bq")
                nc.sync.dma_start(bq_sb[:], bq_r[l])
                bk_sb = wts.tile([1, D4], f32, tag="bk", name="bk")
                nc.sync.dma_start(bk_sb[:], bk_r[l])
                bv_sb = wts.tile([1, D4], f32, tag="bv", name="bv")
                nc.sync.dma_start(bv_sb[:], bv_r[l])
                be_sb = wts.tile([1, D4], f32, tag="be", name="be")
                nc.sync.dma_start(be_sb[:], be_r[l])
                befm_sb = wts.tile([128, H], f32, tag="befm", name="befm")
                nc.sync.dma_start(befm_sb[:], be_fm[l])
                Wmu_sb = [wts.tile([128, D3], f32, tag=f"Wmu{k}", name=f"Wmu{k}")
                          for k in range(3)]
                for k in range(3):
                    nc.sync.dma_start(Wmu_sb[k][:],
                                      Wmu[l, k * 128:(k + 1) * 128, :])
                bmu_sb = wts.tile([1, D3], f32, tag="bmu", name="bmu")
                nc.sync.dma_start(bmu_sb[:], bmu_r[l])
                Wmsg_sb = [wts.tile([128, C], f32, tag=f"Wmsg{k}", name=f"Wmsg{k}")
                           for k in range(3)]
                for k in range(3):
                    nc.sync.dma_start(Wmsg_sb[k][:],
                                      Wmsg[l, k * 128:(k + 1) * 128, :])
                bmsg_sb = wts.tile([1, C], f32, tag="bmsg", name="bmsg")
                nc.sync.dma_start(bmsg_sb[:], bmsg_r[l])
                Wc_sb = [wts.tile([128, 128], f32, tag=f"Wc{k}", name=f"Wc{k}")
                         for k in range(4)]
                for k in range(4):
                    nc.sync.dma_start(Wc_sb[k][:],
                                      Wc_f[l, k * 128:(k + 1) * 128, :])
                bc_sb = wts.tile([1, 128], f32, tag="bc", name="bc")
                nc.sync.dma_start(bc_sb[:], bc_r[l])
                if not host["ln1_trivial"]:
                    g1_sb = wts.tile([P, H * D3], f32, tag="g1", name="g1")
                    nc.sync.dma_start(g1_sb[:],
                                      g1_r[l].to_broadcast([P, H * D3]))
                    b1g_sb = wts.tile([P, H * D3], f32, tag="b1g", name="b1g")
                    nc.sync.dma_start(b1g_sb[:],
                                      b1_r[l].to_broadcast([P, H * D3]))
                if not host["ln2b_trivial"]:
                    wdeg_sb = wts.tile([1, 128], f32, tag="wdeg", name="wdeg")
                    nc.sync.dma_start(wdeg_sb[:], wdeg_r[l])
                    degrow_sb = wts.tile([1, RANGE], f32, tag="degrow", name="degrow")
                    nc.sync.dma_start(degrow_sb[:], deg_row[:])

                for t in range(NT):
                    nft = sb.tile([P, 128], f32, tag="nft", name="nft")
                    nc.sync.dma_start(nft[:], nf_src[t * P:(t + 1) * P, :])
                    pT = pst.tile([P, P], f32, tag="t", name="t")
                    nc.tensor.transpose(pT[:], nft[:], ident[:])
                    nfT = sb.tile([P, P], f32, tag="nfT", name="nfT")
                    nc.vector.tensor_copy(nfT[:], pT[:])
                    pq = ps.tile([P, D4], f32, tag="w", name="w")
                    nc.tensor.matmul(pq[:], lhsT=nfT[:], rhs=Wq_sb[:],
                                     start=True, stop=False, skip_group_check=True)
                    nc.tensor.matmul(pq[:], lhsT=ones_r[:, :P], rhs=bq_sb[:],
                                     start=False, stop=True, skip_group_check=True)
                    pk = ps.tile([P, D4], f32, tag="w", name="w")
                    nc.tensor.matmul(pk[:], lhsT=nfT[:], rhs=Wk_sb[:],
                                     start=True, stop=False, skip_group_check=True)
                    nc.tensor.matmul(pk[:], lhsT=ones_r[:, :P], rhs=bk_sb[:],
                                     start=False, stop=True, skip_group_check=True)
                    pv = ps.tile([P, D4], f32, tag="w", name="w")
                    nc.tensor.matmul(pv[:], lhsT=nfT[:], rhs=Wv_sb[:],
                                     start=True, stop=False, skip_group_check=True)
                    nc.tensor.matmul(pv[:], lhsT=ones_r[:, :P], rhs=bv_sb[:],
                                     start=False, stop=True, skip_group_check=True)
                    q_sb = sb2.tile([P, D4], f32, tag="q_sb", name="q_sb")
                    nc.scalar.copy(q_sb[:], pq[:])
                    k_sb = sb2.tile([P, D4], f32, tag="k_sb", name="k_sb")
                    nc.vector.tensor_copy(k_sb[:], pk[:])
                    v_sb = sb2.tile([P, D4], f32, tag="v_sb", name="v_sb")
                    nc.vector.tensor_copy(v_sb[:], pv[:])
                    qk_sb = sb2.tile([P, D4], f32, tag="qk_sb", name="qk_sb")
                    nc.vector.tensor_tensor(out=qk_sb[:], in0=q_sb[:],
                                            in1=k_sb[:], op=OP.mult)
                    nc.sync.dma_start(q_tab[t * P:(t + 1) * P, :], q_sb[:])
                    nc.sync.dma_start(k_tab[t * P:(t + 1) * P, :], k_sb[:])
                    nc.sync.dma_start(v_tab[t * P:(t + 1) * P, :], v_sb[:])
                    nc.sync.dma_start(qk_tab[t * P:(t + 1) * P, :], qk_sb[:])

                # -- C2/C3: edge messages + scatter + node update --
                for c in range(CHUNKS):
                    qk_ch = sbc.tile([P, D4], f32, tag="qk_ch", name="qk_ch")
                    nc.gpsimd.indirect_dma_start(
                        out=qk_ch[:], out_offset=None, in_=qk_tab[:],
                        in_offset=bass.IndirectOffsetOnAxis(
                            ap=cidx_sb[:, c:c + 1], axis=0))
                    q_ch = sbc.tile([P, D4], f32, tag="q_ch", name="q_ch")
                    nc.gpsimd.indirect_dma_start(
                        out=q_ch[:], out_offset=None, in_=q_tab[:],
                        in_offset=bass.IndirectOffsetOnAxis(
                            ap=cidx_sb[:, c:c + 1], axis=0))
                    v_ch = sbc.tile([P, D4], f32, tag="v_ch", name="v_ch")
                    nc.gpsimd.indirect_dma_start(
                        out=v_ch[:], out_offset=None, in_=v_tab[:],
                        in_offset=bass.IndirectOffsetOnAxis(
                            ap=cidx_sb[:, c:c + 1], axis=0))
                    nf_old = sbc.tile([P, 128], f32, tag="nf_old", name="nf_old")
                    nc.gpsimd.indirect_dma_start(
                        out=nf_old[:], out_offset=None, in_=nf_src[:],
                        in_offset=bass.IndirectOffsetOnAxis(
                            ap=cidx_sb[:, c:c + 1], axis=0))

                    pagg = aggp.tile([P, D4], f32, tag="agg", name="agg")
                    for tt in range(TPC):
                        t = c * TPC + tt
                        # gathers by src
                        ks_g = sb.tile([P, D4], f32, tag="ks_g", name="ks_g")
                        nc.gpsimd.indirect_dma_start(
                            out=ks_g[:], out_offset=None, in_=k_tab[:],
                            in_offset=bass.IndirectOffsetOnAxis(
                                ap=src_sb[:, t:t + 1], axis=0))
                        vs_g = sb.tile([P, D4], f32, tag="vs_g", name="vs_g")
                        nc.gpsimd.indirect_dma_start(
                            out=vs_g[:], out_offset=None, in_=v_tab[:],
                            in_offset=bass.IndirectOffsetOnAxis(
                                ap=src_sb[:, t:t + 1], axis=0))
                        # one-hot (edges x nodes) and transpose
                        oh = sb.tile([P, P], f32, tag="oh", name="oh")
                        nc.vector.tensor_tensor(
                            out=oh[:], in0=doff_sb[:, t:t + 1].to_broadcast(
                                [P, P]), in1=iota_f[:], op=OP.is_equal)
                        poT = pst.tile([P, P], f32, tag="t", name="t")
                        nc.tensor.transpose(poT[:], oh[:], ident[:])
                        oT = sb.tile([P, P], f32, tag="oT", name="oT")
                        nc.vector.tensor_copy(oT[:], poT[:])
                        # dst-side expansions
                        pqkd = ps.tile([P, D4], f32, tag="w", name="w")
                        nc.tensor.matmul(pqkd[:], lhsT=oT[:], rhs=qk_ch[:],
                                         start=True, stop=True, skip_group_check=True)
                        pqd = ps.tile([P, D4], f32, tag="w", name="w")
                        nc.tensor.matmul(pqd[:], lhsT=oT[:], rhs=q_ch[:],
                                         start=True, stop=True, skip_group_check=True)
                        qd_sb = sb.tile([P, D4], f32, tag="qd_sb", name="qd_sb")
                        nc.scalar.copy(qd_sb[:], pqd[:])
                        # v_d feature-major per head
                        vdfm = []
                        for h in range(H):
                            pvd = pst.tile([P, P], f32, tag="t", name="t")
                            nc.tensor.matmul(
                                pvd[:], lhsT=v_ch[:, h * C:(h + 1) * C],
                                rhs=oT[:], start=True, stop=True, skip_group_check=True)
                            vd_h = sb.tile([P, P], f32, tag=f"vdfm{h}", name=f"vdfm{h}")
                            nc.vector.tensor_copy(vd_h[:], pvd[:])
                            vdfm.append(vd_h)
                        # e (edge-major) and e_fm (feature-major)
                        eft = sb.tile([P, P], f32, tag="eft2", name="eft2")
                        nc.sync.dma_start(eft[:],
                                          ef_dram[:, t * P:(t + 1) * P])
                        pe = ps.tile([P, D4], f32, tag="w", name="w")
                        nc.tensor.matmul(pe[:], lhsT=eft[:], rhs=We_sb[:],
                                         start=True, stop=False, skip_group_check=True)
                        nc.tensor.matmul(pe[:], lhsT=ones_r[:, :P],
                                         rhs=be_sb[:], start=False, stop=True, skip_group_check=True)
                        efm = []
                        for h in range(H):
                            pefm = pst.tile([P, P], f32, tag="t", name="t")
                            nc.tensor.matmul(
                                pefm[:], lhsT=We_sb[:, h * C:(h + 1) * C],
                                rhs=eft[:], start=True, stop=True, skip_group_check=True)
                            e_h = sb.tile([P, P], f32, tag=f"efm{h}", name=f"efm{h}")
                            nc.scalar.activation(e_h[:], pefm[:], AF.Identity,
                                                 bias=befm_sb[:, h:h + 1])
                            efm.append(e_h)
                        # v_s feature-major per head (transpose of gather)
                        vsfm = []
                        for h in range(H):
                            pvs = pst.tile([P, P], f32, tag="t", name="t")
                            nc.tensor.transpose(
                                pvs[:], vs_g[:, h * C:(h + 1) * C], ident[:])
                            vs_h = sb.tile([P, P], f32, tag=f"vsfm{h}", name=f"vsfm{h}")
                            nc.vector.tensor_copy(vs_h[:], pvs[:])
                            vsfm.append(vs_h)
                        # alpha [P, H*3C], slot layout per head: [kd, ks, e]
                        alpha = sb2.tile([P, H * D3], f32, tag="alpha", name="alpha")
                        a4 = alpha[:].rearrange("p (h s c) -> p h s c",
                                                h=H, s=3)
                        nc.vector.tensor_copy(
                            a4[:, :, 0, :],
                            pqkd[:].rearrange("p (h c) -> p h c", h=H))
                        nc.vector.tensor_tensor(
                            out=a4[:, :, 1, :],
                            in0=qd_sb[:].rearrange("p (h c) -> p h c", h=H),
                            in1=ks_g[:].rearrange("p (h c) -> p h c", h=H),
                            op=OP.mult)
                        nc.vector.tensor_tensor(
                            out=a4[:, :, 2, :],
                            in0=qd_sb[:].rearrange("p (h c) -> p h c", h=H),
                            in1=pe[:].rearrange("p (h c) -> p h c", h=H),
                            op=OP.mult)
                        # LN1 + sigmoid -> gate
                        su4 = sb.tile([P, H], f32, tag="su4", name="su4")
                        nc.vector.reduce_sum(
                            su4[:], alpha[:].rearrange("p (h f) -> p h f",
                                                       h=H), axis=AX.X)
                        nmu4 = sb.tile([P, H], f32, tag="nmu4", name="nmu4")
                        nc.scalar.mul(nmu4[:], su4[:], -1.0 / D3)
                        vs4 = sb.tile([P, H], f32, tag="vs4", name="vs4")
                        scr = sb.tile([P, D3], f32, tag="scr", name="scr")
                        for h in range(H):
                            nc.scalar.activation(
                                scr[:], alpha[:, h * D3:(h + 1) * D3],
                                AF.Square, bias=nmu4[:, h:h + 1],
                                accum_out=vs4[:, h:h + 1])
                        v4 = sb.tile([P, H], f32, tag="v4", name="v4")
                        nc.vector.tensor_scalar(out=v4[:], in0=vs4[:],
                                                scalar1=1.0 / D3,
                                                scalar2=EPS, op0=OP.mult,
                                                op1=OP.add)
                        rstd4 = rsqrt(v4[:], [P, H], nc.vector)
                        nmr4 = sb.tile([P, H], f32, tag="nmr4", name="nmr4")
                        nc.vector.tensor_tensor(out=nmr4[:], in0=nmu4[:],
                                                in1=rstd4, op=OP.mult)
                        gate = sb2.tile([P, H * D3], f32, tag="gate", name="gate")
                        if host["ln1_trivial"]:
                            for h in range(H):
                                nc.scalar.activation(
                                    gate[:, h * D3:(h + 1) * D3],
                                    alpha[:, h * D3:(h + 1) * D3],
                                    AF.Sigmoid, scale=rstd4[:, h:h + 1],
                                    bias=nmr4[:, h:h + 1])
                        else:
                            xn = sb2.tile([P, H * D3], f32, tag="xn", name="xn")
                            for h in range(H):
                                nc.scalar.activation(
                                    xn[:, h * D3:(h + 1) * D3],
                                    alpha[:, h * D3:(h + 1) * D3],
                                    AF.Identity, scale=rstd4[:, h:h + 1],
                                    bias=nmr4[:, h:h + 1])
                            nc.vector.tensor_tensor(out=xn[:], in0=xn[:],
                                                    in1=g1_sb[:], op=OP.mult)
                            nc.vector.tensor_tensor(out=xn[:], in0=xn[:],
                                                    in1=b1g_sb[:], op=OP.add)
                            nc.scalar.activation(gate[:], xn[:], AF.Sigmoid)
                        # m1 = m_in @ Wmu + bmu ; m1g = m1 * gate
                        m1g = sb2.tile([P, H * D3], f32, tag="m1g", name="m1g")
                        for h in range(H):
                            pm1 = ps.tile([P, D3], f32, tag="w", name="w")
                            nc.tensor.matmul(pm1[:], lhsT=vdfm[h][:],
                                             rhs=Wmu_sb[0][:],
                                             start=True, stop=False, skip_group_check=True)
                            nc.tensor.matmul(pm1[:], lhsT=vsfm[h][:],
                                             rhs=Wmu_sb[1][:],
                                             start=False, stop=False, skip_group_check=True)
                            nc.tensor.matmul(pm1[:], lhsT=efm[h][:],
                                             rhs=Wmu_sb[2][:],
                                             start=False, stop=False, skip_group_check=True)
                            nc.tensor.matmul(pm1[:], lhsT=ones_r[:, :P],
                                             rhs=bmu_sb[:],
                                             start=False, stop=True, skip_group_check=True)
                            nc.vector.tensor_tensor(
                                out=m1g[:, h * D3:(h + 1) * D3], in0=pm1[:],
                                in1=gate[:, h * D3:(h + 1) * D3], op=OP.mult)
                        # m2 = m1g @ Wmsg + bmsg ; LN2 -> m_fin
                        # (per-head matmul, psum copied out fast; batched
                        #  small-op LN math)
                        m_fin = sb2.tile([P, D4], f32, tag="m_fin", name="m_fin")
                        m2sb = sb2.tile([P, D4], f32, tag="m2sb", name="m2sb")
                        s24 = sb.tile([P, H], f32, tag="s24", name="s24")
                        for h in range(H):
                            pm2 = ps.tile([P, C], f32, tag="w", name="w")
                            for k in range(3):
                                pmT = pst.tile([P, P], f32, tag="t", name="t")
                                nc.tensor.transpose(
                                    pmT[:],
                                    m1g[:, h * D3 + k * 128:
                                        h * D3 + (k + 1) * 128], ident[:])
                                mT = sb.tile([P, P], f32, tag="mT", name="mT")
                                nc.vector.tensor_copy(mT[:], pmT[:])
                                nc.tensor.matmul(pm2[:], lhsT=mT[:],
                                                 rhs=Wmsg_sb[k][:],
                                                 start=(k == 0), stop=False,
                                                 skip_group_check=True)
                            nc.tensor.matmul(pm2[:], lhsT=ones_r[:, :P],
                                             rhs=bmsg_sb[:],
                                             start=False, stop=True,
                                             skip_group_check=True)
                            nc.vector.tensor_copy(
                                m2sb[:, h * C:(h + 1) * C], pm2[:])
                            nc.vector.reduce_sum(
                                s24[:, h:h + 1], pm2[:], axis=AX.X)
                        nmu24 = sb.tile([P, H], f32, tag="nmu24", name="nmu24")
                        nc.scalar.mul(nmu24[:], s24[:], -1.0 / C)
                        vs24 = sb.tile([P, H], f32, tag="vs24", name="vs24")
                        scr2 = sb.tile([P, C], f32, tag="scr2", name="scr2")
                        for h in range(H):
                            nc.scalar.activation(
                                scr2[:], m2sb[:, h * C:(h + 1) * C],
                                AF.Square, bias=nmu24[:, h:h + 1],
                                accum_out=vs24[:, h:h + 1])
                        v24 = sb.tile([P, H], f32, tag="v24", name="v24")
                        nc.vector.tensor_scalar(out=v24[:], in0=vs24[:],
                                                scalar1=1.0 / C, scalar2=EPS,
                                                op0=OP.mult, op1=OP.add)
                        rstd24 = rsqrt(v24[:], [P, H], nc.vector)
                        nmr24 = sb.tile([P, H], f32, tag="nmr24", name="nmr24")
                        nc.vector.tensor_tensor(out=nmr24[:], in0=nmu24[:],
                                                in1=rstd24, op=OP.mult)
                        for h in range(H):
                            nc.scalar.activation(
                                m_fin[:, h * C:(h + 1) * C],
                                m2sb[:, h * C:(h + 1) * C],
                                AF.Identity, scale=rstd24[:, h:h + 1],
                                bias=nmr24[:, h:h + 1])
                        # scatter-add into chunk aggregate
                        nc.tensor.matmul(pagg[:], lhsT=oh[:], rhs=m_fin[:],
                                         start=(tt == 0), stop=(tt == TPC - 1),
                                         skip_group_check=True)

                    # -- node update for chunk c --
                    agg_sb = sb2.tile([P, D4], f32, tag="agg_sb", name="agg_sb")
                    nc.vector.tensor_copy(agg_sb[:], pagg[:])
                    pout = ps.tile([P, 128], f32, tag="w", name="w")
                    for k in range(4):
                        paT = pst.tile([P, P], f32, tag="t", name="t")
                        nc.tensor.transpose(
                            paT[:], agg_sb[:, k * 128:(k + 1) * 128],
                            ident[:])
                        aT = sb.tile([P, P], f32, tag="aT", name="aT")
                        nc.vector.tensor_copy(aT[:], paT[:])
                        nc.tensor.matmul(pout[:], lhsT=aT[:], rhs=Wc_sb[k][:],
                                         start=(k == 0), stop=False, skip_group_check=True)
                    last = host["ln2b_trivial"]
                    nc.tensor.matmul(pout[:], lhsT=ones_r[:, :P],
                                     rhs=bc_sb[:], start=False, stop=last, skip_group_check=True)
                    if not host["ln2b_trivial"]:
                        nc.tensor.matmul(
                            pout[:],
                            lhsT=degrow_sb[:, c * P:(c + 1) * P],
                            rhs=wdeg_sb[:], start=False, stop=True, skip_group_check=True)
                    nfn = sb.tile([P, 128], f32, tag="nfn", name="nfn")
                    nc.vector.tensor_tensor(out=nfn[:], in0=pout[:],
                                            in1=nf_old[:], op=OP.add)
                    sgx = sb.tile([P, 128], f32, tag="sgx", name="sgx")
                    nc.scalar.activation(sgx[:], nfn[:], AF.Sigmoid)
                    nfo = sb.tile([P, 128], f32, tag="nfo", name="nfo")
                    nc.vector.tensor_tensor(out=nfo[:], in0=nfn[:],
                                            in1=sgx[:], op=OP.mult)
                    nc.sync.dma_start(ag_in[l][c * P:(c + 1) * P, :], nfo[:])

                # -- C4: allgather nf --
                nc.gpsimd.collective_compute(
                    "AllGather", OP.bypass,
                    ins=[ag_in[l].opt()],
                    outs=[ag_out[l].opt()],
                    replica_groups=[list(range(NCORES))],
                )

            # ---------------- pooling + readout ----------------
            gidA_sb = cpool.tile([P, CHUNKS], f32, name="t120")
            nc.sync.dma_start(gidA_sb[:], gidA_T[:])
            gidB_sb = cpool.tile([P, CHUNKS], f32, name="t121")
            nc.sync.dma_start(gidB_sb[:], gidB_T[:])
            nf_fin = ag_in[nl_build - 1]
            ppA = aggp.tile([P, 129], f32, tag="agg", name="agg")
            ppB = aggp.tile([P, 129], f32, tag="agg", name="agg")
            for c in range(CHUNKS):
                rhs = sb.tile([P, 129], f32, tag="prhs", name="prhs")
                nc.sync.dma_start(rhs[:, :128],
                                  nf_fin[c * P:(c + 1) * P, :])
                nc.gpsimd.memset(rhs[:, 128:129], 1.0)
                ohA = sb.tile([P, P], f32, tag="ohA", name="ohA")
                nc.vector.tensor_tensor(
                    out=ohA[:], in0=gidA_sb[:, c:c + 1].to_broadcast([P, P]),
                    in1=iota_f[:], op=OP.is_equal)
                ohB = sb.tile([P, P], f32, tag="ohB", name="ohB")
                nc.vector.tensor_tensor(
                    out=ohB[:], in0=gidB_sb[:, c:c + 1].to_broadcast([P, P]),
                    in1=iota_f[:], op=OP.is_equal)
                nc.tensor.matmul(ppA[:], lhsT=ohA[:], rhs=rhs[:],
                                 start=(c == 0), stop=(c == CHUNKS - 1),
                                 skip_group_check=True)
                nc.tensor.matmul(ppB[:], lhsT=ohB[:], rhs=rhs[:],
                                 start=(c == 0), stop=(c == CHUNKS - 1),
                                 skip_group_check=True)
            pA_sb = sb.tile([P, 129], f32, tag="pA_sb", name="pA_sb")
            nc.vector.tensor_copy(pA_sb[:], ppA[:])
            pB_sb = sb.tile([P, 129], f32, tag="pB_sb", name="pB_sb")
            nc.vector.tensor_copy(pB_sb[:], ppB[:])
            nc.sync.dma_start(ar_in[0:P, :], pA_sb[:])
            nc.sync.dma_start(ar_in[P:2 * P, :], pB_sb[:])
            nc.gpsimd.collective_compute(
                "AllReduce", OP.add,
                ins=[ar_in.opt()],
                outs=[ar_out.opt()],
                replica_groups=[list(range(NCORES))],
            )
            fcW_sb = cpool.tile([128, 128], f32, name="t129")
            nc.sync.dma_start(fcW_sb[:], fc_W[:])
            fcb_sb = cpool.tile([1, 128], f32, name="t130")
            nc.sync.dma_start(fcb_sb[:], fc_b_r[:])
            outW_sb = cpool.tile([128, 1], f32, name="t131")
            nc.sync.dma_start(outW_sb[:], out_W[:])
            for half in range(2):
                pool_t = sb.tile([P, 129], f32, tag="pool_t", name="pool_t")
                nc.sync.dma_start(pool_t[:],
                                  ar_out[half * P:(half + 1) * P, :])
                cnt = sb.tile([P, 1], f32, tag="cnt", name="cnt")
                nc.vector.tensor_scalar_max(cnt[:], pool_t[:, 128:129], 1.0)
                rc = sb.tile([P, 1], f32, tag="rc", name="rc")
                nc.vector.reciprocal(rc[:], cnt[:])
                pooled = sb.tile([P, 128], f32, tag="pooled", name="pooled")
                nc.vector.tensor_scalar_mul(pooled[:], pool_t[:, :128],
                                            rc[:, :1])
                ppT = pst.tile([P, P], f32, tag="t", name="t")
                nc.tensor.transpose(ppT[:], pooled[:], ident[:])
                poolT = sb.tile([P, P], f32, tag="poolT", name="poolT")
                nc.vector.tensor_copy(poolT[:], ppT[:])
                pf = ps.tile([P, 128], f32, tag="w", name="w")
                nc.tensor.matmul(pf[:], lhsT=poolT[:], rhs=fcW_sb[:],
                                 start=True, stop=False, skip_group_check=True)
                nc.tensor.matmul(pf[:], lhsT=ones_r[:, :P], rhs=fcb_sb[:],
                                 start=False, stop=True, skip_group_check=True)
                sgf = sb.tile([P, 128], f32, tag="sgf", name="sgf")
                nc.scalar.activation(sgf[:], pf[:], AF.Sigmoid)
                feats = sb.tile([P, 128], f32, tag="feats", name="feats")
                nc.vector.tensor_tensor(out=feats[:], in0=pf[:],
                                        in1=sgf[:], op=OP.mult)
                pfT = pst.tile([P, P], f32, tag="t", name="t")
                nc.tensor.transpose(pfT[:], feats[:], ident[:])
                featT = sb.tile([P, P], f32, tag="featT", name="featT")
                nc.vector.tensor_copy(featT[:], pfT[:])
                po = ps.tile([P, 1], f32, tag="w", name="w")
                nc.tensor.matmul(po[:], lhsT=featT[:], rhs=outW_sb[:],
                                 start=True, stop=True, skip_group_check=True)
                yt = sb.tile([P, 1], f32, tag="yt", name="yt")
                nc.vector.tensor_scalar_add(yt[:], po[:],
                                            host["out_b_val"])
                nc.sync.dma_start(y[half * P:(half + 1) * P, :], yt[:])

    nc.finalize()
    return nc


def _in_maps(host):
    shared = {k: host[k] for k in
              ["x_fm", "emb_W", "emb_b_r", "rbf_W1", "rbf_W2", "b1_col",
               "b2_col", "centers_r", "Wq_f", "bq_r", "Wk", "bk_r", "Wv",
               "bv_r", "We", "be_r", "be_fm", "Wmu", "bmu_r", "Wmsg",
               "bmsg_r", "Wc_f", "bc_r", "fc_W", "fc_b_r", "out_W"]}
    if not host["ln1_trivial"]:
        shared["g1_r"] = host["g1_r"]
        shared["b1_r"] = host["b1_r"]
    maps = []
    for i in range(NCORES):
        pc = host["percore"][i]
        m = dict(shared)
        m["src_T"] = pc["src_T"]
        m["doff_T"] = pc["doff_T"]
        m["ea_s"] = pc["ea_s"]
        m["chunk_idx_T"] = pc["chunk_idx_T"]
        m["gidA_T"] = pc["gidA_T"]
        m["gidB_T"] = pc["gidB_T"]
        if not host["ln2b_trivial"]:
            m["wdeg_r"] = host["wdeg_r"]
            m["deg_row"] = pc["deg_row"]
        maps.append(m)
    return maps


def kernel(_trace=False, **inputs):
    from concourse import bass_utils
    host = _prep(inputs)
    nc = _build(host, nl_build=_NL_BUILD)
    res = bass_utils.run_bass_kernel_spmd(
        nc, _in_maps(host), core_ids=list(range(NCORES)), trace=_trace)
    y = np.asarray(res.results[0]["y"])[:, 0].astype(np.float32)
    if _trace:
        kernel.last_result = res
    return y



# revision 11
# speedup vs baseline: 2.3926x; 2.3926x over previous
"""Matformer GNN message-passing kernel for 8 Trainium2 NeuronCores.

Sharding: nodes in contiguous ranges of 1280 per core (batch is sorted so
this approximates graph sharding); edges sharded by dst node range and
grouped by 128-node chunk. Edge message compute (the dominant cost) is
fully sharded; node-level projections are replicated (cheap); nf is
all-gathered between layers; pooled sums are all-reduced at the end.

bf16 datapath (fp32 matmuls run LOW_HIGH = 4 cycles/row; bf16 runs 1);
the residual stream nf stays fp32 for accuracy. PSUM stays fp32; LN
statistics accumulate in fp32. One-hot scatter/expand matrices are
host-precomputed. Bias adds are folded into weight tables (A-table,
Ce-fold) or into PSUM-evacuation tensor_tensor adds — almost no K=1
bias matmuls remain. Edge capacity per chunk is data-driven.
"""
import numpy as np
import ml_dtypes

BF16 = np.float16  # fp16: same 1-cyc/row PE speed as bf16,
# but 10-bit mantissa — bf16 weight rounding alone cost 2.7e-2 rel err

# ---- problem constants (hardcoded per contest rules) ----
N, E, G = 10000, 100000, 256
H, C = 4, 128
NL = 5
NCORES = 8
P = 128
RANGE = 1280                 # nodes per core
NPAD = RANGE * NCORES        # 10240
NT = NPAD // P               # 80 node tiles
CHUNKS = RANGE // P          # 10 chunks per core
D4 = H * C                   # 512
D3 = 3 * C                   # 384
GAMMA = 1.0 / (8.0 / 127.0)
INV_SQRT = 1.0 / np.sqrt(3.0 * C)
EPS = 1e-5
BN_S = 1.0 / np.sqrt(1.0 + 1e-5)

_NL_BUILD = NL  # overridable for compile-time experiments


def _prep(inp):
    """Host-side data movement: shard + sort edges, fold constants."""
    f32 = np.float32
    x = np.asarray(inp["x"], f32)
    edge_attr = np.asarray(inp["edge_attr"], f32)
    edge_index = np.asarray(inp["edge_index"]).astype(np.int64)
    batch = np.asarray(inp["batch"]).astype(np.int64)
    src, dst = edge_index[0], edge_index[1]

    host = {}
    # ---- weights (shared across cores) ----
    host["x_fm"] = np.zeros((92, NPAD), f32)
    host["x_fm"][:, :N] = x.T
    host["emb_W"] = np.asarray(inp["emb_W"], f32)
    host["emb_b_r"] = np.asarray(inp["emb_b"], f32).reshape(1, 128)
    host["rbf_W1"] = np.asarray(inp["rbf_W1"], f32).astype(BF16)
    host["rbf_W2"] = np.asarray(inp["rbf_W2"], f32).astype(BF16)
    host["b1_col"] = np.asarray(inp["rbf_b1"], f32).reshape(128, 1)
    host["b2_col"] = np.asarray(inp["rbf_b2"], f32).reshape(128, 1)
    host["centers_r"] = np.linspace(0.0, 8.0, 128, dtype=f32).reshape(1, 128)

    Wq = np.asarray(inp["Wq"], f32) * INV_SQRT
    bq = np.asarray(inp["bq"], f32) * INV_SQRT
    host["Wq_f"] = Wq.astype(BF16)
    host["bq_r"] = bq.reshape(NL, 1, D4).astype(BF16)
    host["Wk"] = np.asarray(inp["Wk"], f32).astype(BF16)
    host["bk_r"] = np.asarray(inp["bk"], f32).reshape(NL, 1, D4).astype(BF16)
    host["Wv"] = np.asarray(inp["Wv"], f32).astype(BF16)
    host["bv_r"] = np.asarray(inp["bv"], f32).reshape(NL, 1, D4).astype(BF16)
    We_f = np.asarray(inp["We"], f32)
    be_f = np.asarray(inp["be"], f32)
    host["We"] = We_f.astype(BF16)
    host["be_r"] = be_f.reshape(NL, 1, D4).astype(BF16)
    Wmu_f = np.asarray(inp["Wmu"], f32)
    bmu_f = np.asarray(inp["bmu"], f32)
    host["Wmu"] = Wmu_f.astype(BF16)
    Wmsg_f = np.asarray(inp["Wmsg"], f32)
    bmsg_f = np.asarray(inp["bmsg"], f32)

    # Ce-fold: e@Wmu2 = ef@(We_h@Wmu2) + be_h@Wmu2.  Wfold[l] has the four
    # per-head folds side by side: [128, H*D3].
    Wfold = np.zeros((NL, 128, H * D3), f32)
    bmua = np.zeros((NL, 1, H * D3), f32)
    for l in range(NL):
        Wmu2 = Wmu_f[l, 2 * C:, :]                     # [C, D3]
        for h in range(H):
            Wfold[l, :, h * D3:(h + 1) * D3] = \
                We_f[l][:, h * C:(h + 1) * C] @ Wmu2
            bmua[l, 0, h * D3:(h + 1) * D3] = \
                bmu_f[l] + be_f[l, h * C:(h + 1) * C] @ Wmu2
    host["Wfold"] = Wfold.astype(BF16)
    host["bmua_r"] = bmua.astype(BF16)

    host["Wmsg"] = Wmsg_f.astype(BF16)
    host["bmsg4_r"] = np.tile(bmsg_f, (1, H)).reshape(NL, 1, D4).astype(BF16)

    ln1_g = np.asarray(inp["ln1_g"], f32)
    ln1_b = np.asarray(inp["ln1_b"], f32)
    ln2_g = np.asarray(inp["ln2_g"], f32)
    ln2_b = np.asarray(inp["ln2_b"], f32)
    host["ln1_trivial"] = bool(
        np.allclose(ln1_g, 1.0) and np.allclose(ln1_b, 0.0))
    host["g1_r"] = np.tile(ln1_g, (1, H)).reshape(NL, 1, H * D3).astype(BF16)
    host["b1_r"] = np.tile(ln1_b, (1, H)).reshape(NL, 1, H * D3).astype(BF16)

    Wc = np.asarray(inp["Wc"], f32)
    bc = np.asarray(inp["bc"], f32)
    bn_g = np.asarray(inp["bn_g"], f32)
    bn_b = np.asarray(inp["bn_b"], f32)
    colscale = BN_S * bn_g                       # [NL,128]
    rowscale = np.tile(ln2_g, (1, H))            # [NL,512]
    host["Wc_f"] = (Wc * rowscale[:, :, None] * colscale[:, None, :]
                    ).astype(BF16)
    host["bc_r"] = (bc * colscale + bn_b).reshape(NL, 1, 128).astype(BF16)
    host["ln2b_trivial"] = bool(np.allclose(ln2_b, 0.0))
    wdeg = np.einsum("lk,lkj->lj", np.tile(ln2_b, (1, H)), Wc) * colscale
    host["wdeg_r"] = wdeg.reshape(NL, 1, 128).astype(BF16)

    host["fc_W"] = np.asarray(inp["fc_W"], f32)
    host["fc_b_r"] = np.asarray(inp["fc_b"], f32).reshape(1, 128)
    host["out_W"] = np.asarray(inp["out_W"], f32)
    host["out_b_val"] = float(np.asarray(inp["out_b"], f32).reshape(-1)[0])

    # ---- per-core edge sharding (data-driven per-chunk capacity) ----
    maxcnt = 0
    for i in range(NCORES):
        lo = RANGE * i
        for c in range(CHUNKS):
            nlo = lo + c * P
            cnt = int(np.sum((dst >= nlo) & (dst < min(nlo + P, N))))
            maxcnt = max(maxcnt, cnt)
    TPC = max((maxcnt + P - 1) // P, 1)
    ECAP_CHUNK = TPC * P
    ET = CHUNKS * TPC
    ECAP = ET * P
    host["TPC"], host["ET"], host["ECAP"] = TPC, ET, ECAP

    deg = np.bincount(dst, minlength=NPAD).astype(f32)
    percore = []
    for i in range(NCORES):
        lo, hi = RANGE * i, RANGE * (i + 1)
        src_T = np.zeros((P, ET), np.int32)
        ea_s = np.zeros((ECAP, 3), f32)
        oh_all = np.zeros((P, ET * P), BF16)
        oT_all = np.zeros((P, ET * P), BF16)
        for c in range(CHUNKS):
            nlo = lo + c * P
            sel = np.nonzero((dst >= nlo) & (dst < min(nlo + P, N)))[0]
            cnt = len(sel)
            base = c * ECAP_CHUNK
            flat_t = (base + np.arange(cnt)) // P
            flat_p = (base + np.arange(cnt)) % P
            src_T[flat_p, flat_t] = src[sel]
            doffc = (dst[sel] - nlo).astype(np.int64)
            oh_all[flat_p, flat_t * P + doffc] = 1
            oT_all[doffc, flat_t * P + flat_p] = 1
            ea_s[base:base + cnt] = edge_attr[sel]
        chunk_idx_T = (lo + np.arange(RANGE).reshape(CHUNKS, P).T
                       ).astype(np.int32)            # [P, CHUNKS]
        gid = np.full(RANGE, 999.0, f32)
        nreal = min(hi, N) - lo
        if nreal > 0:
            gid[:nreal] = batch[lo:lo + nreal].astype(f32)
        gidA_T = gid.reshape(CHUNKS, P).T.copy()
        gidB_T = (gid - 128.0).reshape(CHUNKS, P).T.copy()
        deg_row = np.zeros((1, RANGE), BF16)
        deg_row[0, :nreal] = deg[lo:lo + nreal].astype(BF16)
        percore.append(dict(src_T=src_T, ea_s=ea_s, oh_all=oh_all,
                            oT_all=oT_all, chunk_idx_T=chunk_idx_T,
                            gidA_T=gidA_T, gidB_T=gidB_T, deg_row=deg_row))
    host["percore"] = percore
    return host


def _build(host, nl_build=NL):
    import concourse.bacc as bacc
    import concourse.tile as tile
    from concourse import bass, mybir
    from concourse.masks import make_identity

    f32 = mybir.dt.float32
    bf16 = mybir.dt.float16
    i32 = mybir.dt.int32
    AF = mybir.ActivationFunctionType
    OP = mybir.AluOpType
    AX = mybir.AxisListType

    TPC, ET, ECAP = host["TPC"], host["ET"], host["ECAP"]

    nc = bacc.Bacc("TRN2", target_bir_lowering=False, debug=False,
                   enable_asserts=False, num_devices=NCORES)

    def din(name, shape, dt=bf16):
        return nc.dram_tensor(name, list(shape), dt, kind="ExternalInput")

    # weights
    x_fm = din("x_fm", (92, NPAD), f32)
    emb_W = din("emb_W", (92, 128), f32)
    emb_b_r = din("emb_b_r", (1, 128), f32)
    rbf_W1 = din("rbf_W1", (128, 128))
    rbf_W2 = din("rbf_W2", (128, 128))
    b1_col = din("b1_col", (128, 1), f32)
    b2_col = din("b2_col", (128, 1), f32)
    centers_r = din("centers_r", (1, 128), f32)
    Wq_f = din("Wq_f", (NL, 128, D4))
    bq_r = din("bq_r", (NL, 1, D4))
    Wk = din("Wk", (NL, 128, D4))
    bk_r = din("bk_r", (NL, 1, D4))
    Wv = din("Wv", (NL, 128, D4))
    bv_r = din("bv_r", (NL, 1, D4))
    We = din("We", (NL, 128, D4))
    be_r = din("be_r", (NL, 1, D4))
    Wmu = din("Wmu", (NL, D3, D3))
    Wfold = din("Wfold", (NL, 128, H * D3))
    bmua_r = din("bmua_r", (NL, 1, H * D3))
    Wmsg = din("Wmsg", (NL, D3, C))
    bmsg4_r = din("bmsg4_r", (NL, 1, D4))
    Wc_f = din("Wc_f", (NL, D4, 128))
    bc_r = din("bc_r", (NL, 1, 128))
    fc_W = din("fc_W", (128, 128), f32)
    fc_b_r = din("fc_b_r", (1, 128), f32)
    out_W = din("out_W", (128, 1), f32)
    if not host["ln1_trivial"]:
        g1_r = din("g1_r", (NL, 1, H * D3))
        b1_r = din("b1_r", (NL, 1, H * D3))
    if not host["ln2b_trivial"]:
        wdeg_r = din("wdeg_r", (NL, 1, 128))
        deg_row = din("deg_row", (1, RANGE))
    # per-core data
    src_T = din("src_T", (P, ET), i32)
    ea_s = din("ea_s", (ECAP, 3), f32)
    oh_all = din("oh_all", (P, ET * P))
    oT_all = din("oT_all", (P, ET * P))
    chunk_idx_T = din("chunk_idx_T", (P, CHUNKS), i32)
    gidA_T = din("gidA_T", (P, CHUNKS), f32)
    gidB_T = din("gidB_T", (P, CHUNKS), f32)

    y = nc.dram_tensor("y", [G, 1], f32, kind="ExternalOutput")

    with tile.TileContext(nc) as tc:
        with tc.tile_pool(name="const", bufs=1) as cpool, \
             tc.tile_pool(name="dram", bufs=1, space="DRAM") as dram, \
             tc.tile_pool(name="wts", bufs=1) as wts, \
             tc.tile_pool(name="sb", bufs=4) as sb, \
             tc.tile_pool(name="sb2", bufs=3) as sb2, \
             tc.tile_pool(name="sbc", bufs=2) as sbc, \
             tc.tile_pool(name="ps", bufs=4, space="PSUM") as ps, \
             tc.tile_pool(name="pst", bufs=2, space="PSUM") as pst, \
             tc.tile_pool(name="agg", bufs=2, space="PSUM") as aggp:

            # ---------------- constants ----------------
            ident = cpool.tile([P, P], bf16, name="t0")
            make_identity(nc, ident[:])
            identf = cpool.tile([P, P], f32, name="t0f")
            make_identity(nc, identf[:])
            iota_i = cpool.tile([P, P], i32, name="t1")
            nc.gpsimd.iota(iota_i[:], pattern=[[1, P]], base=0,
                           channel_multiplier=0)
            iota_f = cpool.tile([P, P], f32, name="t2")
            nc.vector.tensor_copy(iota_f[:], iota_i[:])
            ones_r = cpool.tile([1, D4], bf16, name="t4")
            nc.gpsimd.memset(ones_r[:], 1.0)
            zero_r = cpool.tile([1, D4], bf16, name="t4z")
            nc.gpsimd.memset(zero_r[:], 0.0)
            onesf = cpool.tile([1, P], f32, name="t4f")
            nc.gpsimd.memset(onesf[:], 1.0)
            centers_b = cpool.tile([P, P], f32, name="t5")
            nc.sync.dma_start(centers_b[:], centers_r[:].to_broadcast([P, P]))

            _rs_uid = [0]

            def rsqrt(x_ap, shape, eng, iters=2):
                """rsqrt(x) via quake bit-trick + Newton, all on `eng`
                (no ACT table needed). x must be > 0."""
                u = _rs_uid[0]
                _rs_uid[0] += 1
                ish = sb.tile(shape, i32, tag="rs_sh", name=f"rsh{u}")
                eng.tensor_scalar(out=ish[:], in0=x_ap.bitcast(i32),
                                  scalar1=1, scalar2=None,
                                  op0=OP.logical_shift_right)
                y0 = sb.tile(shape, i32, tag="rs_y0", name=f"rsy{u}")
                eng.tensor_scalar(out=y0[:], in0=ish[:], scalar1=-1,
                                  scalar2=0x5f3759df, op0=OP.mult,
                                  op1=OP.add)
                hv = sb.tile(shape, f32, tag="rs_hv", name=f"rsh2{u}")
                eng.tensor_scalar(out=hv[:], in0=x_ap, scalar1=-0.5,
                                  scalar2=None, op0=OP.mult)
                y = y0[:].bitcast(f32)
                for it in range(iters):
                    y2 = sb.tile(shape, f32, tag=f"rs_a{it}",
                                 name=f"rsa{u}_{it}")
                    eng.tensor_tensor(out=y2[:], in0=y, in1=y, op=OP.mult)
                    t2 = sb.tile(shape, f32, tag=f"rs_b{it}",
                                 name=f"rsb{u}_{it}")
                    eng.tensor_tensor(out=t2[:], in0=y2[:], in1=hv[:],
                                      op=OP.mult)
                    yn = sb.tile(shape, f32, tag=f"rs_c{it}",
                                 name=f"rsc{u}_{it}")
                    eng.scalar_tensor_tensor(out=yn[:], in0=t2[:],
                                             scalar=1.5, in1=y,
                                             op0=OP.add, op1=OP.mult)
                    y = yn[:]
                return y

            # persistent per-core edge metadata
            src_sb = cpool.tile([P, ET], i32, name="t6")
            nc.sync.dma_start(src_sb[:], src_T[:])
            cidx_sb = cpool.tile([P, CHUNKS], i32, name="t8")
            nc.sync.dma_start(cidx_sb[:], chunk_idx_T[:])

            # ---------------- DRAM scratch ----------------
            nf0 = dram.tile([NPAD, 128], f32, name="t9")
            kv_tab = dram.tile([NPAD, 2 * D4], bf16, name="t10")
            qkq_tab = dram.tile([NPAD, 2 * D4], bf16, name="t11")
            ef_dram = dram.tile([128, ECAP], bf16, name="t14")
            ag_in = [dram.tile([RANGE, 128], f32, name="t15")
                     for _ in range(nl_build)]
            ag_out = [dram.tile([NPAD, 128], f32, addr_space="Shared",
                                name="t16") for _ in range(nl_build)]
            ar_in = dram.tile([2 * P, 129], f32, name="t17")
            ar_out = dram.tile([2 * P, 129], f32, addr_space="Shared",
                               name="t18")

            # ---------------- stage A: nf0 = x @ emb_W + emb_b ------------
            embW_sb = cpool.tile([92, 128], f32, name="t19")
            nc.sync.dma_start(embW_sb[:], emb_W[:])
            embb_sb = cpool.tile([1, 128], f32, name="t20")
            nc.sync.dma_start(embb_sb[:], emb_b_r[:])
            for t in range(NT):
                xt = sb.tile([92, P], f32, tag="xt", name="xt")
                nc.sync.dma_start(xt[:], x_fm[:, t * P:(t + 1) * P])
                pnf = ps.tile([P, 128], f32, tag="w", name="w")
                nc.tensor.matmul(pnf[:], lhsT=xt[:], rhs=embW_sb[:],
                                 start=True, stop=False, skip_group_check=True)
                nc.tensor.matmul(pnf[:], lhsT=onesf[:], rhs=embb_sb[:],
                                 start=False, stop=True, skip_group_check=True)
                nft = sb.tile([P, 128], f32, tag="nft0", name="nft0")
                nc.vector.tensor_copy(nft[:], pnf[:])
                nc.sync.dma_start(nf0[t * P:(t + 1) * P, :], nft[:])

            # ---------------- stage B: ef (feature-major) -----------------
            W1_sb = cpool.tile([128, 128], bf16, name="t24")
            nc.sync.dma_start(W1_sb[:], rbf_W1[:])
            W2_sb = cpool.tile([128, 128], bf16, name="t25")
            nc.sync.dma_start(W2_sb[:], rbf_W2[:])
            b1_sb = cpool.tile([128, 1], f32, name="t26")
            nc.sync.dma_start(b1_sb[:], b1_col[:])
            b2_sb = cpool.tile([128, 1], f32, name="t27")
            nc.sync.dma_start(b2_sb[:], b2_col[:])
            for t in range(ET):
                ea_t = sb.tile([P, 3], f32, tag="ea", name="ea")
                nc.sync.dma_start(ea_t[:], ea_s[t * P:(t + 1) * P, :])
                ssq = sb.tile([P, 1], f32, tag="ssq", name="ssq")
                sq3 = sb.tile([P, 3], f32, tag="sq3", name="sq3")
                nc.scalar.activation(sq3[:], ea_t[:], AF.Square,
                                     accum_out=ssq[:, :1])
                ssq_e = sb.tile([P, 1], f32, tag="ssq_e", name="ssq_e")
                nc.vector.tensor_scalar(out=ssq_e[:], in0=ssq[:],
                                        scalar1=1e-30, scalar2=None,
                                        op0=OP.add)
                rsq = rsqrt(ssq_e[:], [P, 1], nc.vector)
                dlen = sb.tile([P, 1], f32, tag="dlen", name="dlen")
                nc.vector.tensor_tensor(out=dlen[:], in0=ssq[:], in1=rsq,
                                        op=OP.mult)
                diff = sb.tile([P, P], f32, tag="diff", name="diff")
                nc.vector.tensor_scalar(out=diff[:], in0=centers_b[:],
                                        scalar1=dlen[:, :1], scalar2=None,
                                        op0=OP.subtract)
                dsq = sb.tile([P, P], f32, tag="dsq", name="dsq")
                nc.scalar.square(dsq[:], diff[:])
                rbf = sb.tile([P, P], bf16, tag="rbf", name="rbf")
                nc.scalar.activation(rbf[:], dsq[:], AF.Exp, scale=-GAMMA)
                prT = pst.tile([P, P], bf16, tag="t", name="t")
                nc.tensor.transpose(prT[:], rbf[:], ident[:])
                rbfT = sb.tile([P, P], bf16, tag="rbfT", name="rbfT")
                nc.vector.tensor_copy(rbfT[:], prT[:])
                ph = ps.tile([P, P], f32, tag="w", name="w")
                nc.tensor.matmul(ph[:], lhsT=W1_sb[:], rhs=rbfT[:],
                                 start=True, stop=True, skip_group_check=True)
                eh = sb.tile([P, P], bf16, tag="eh", name="eh")
                nc.scalar.activation(eh[:], ph[:], AF.Exp,
                                     bias=b1_sb[:, :1])
                hfm = sb.tile([P, P], bf16, tag="hfm", name="hfm")
                nc.scalar.activation(hfm[:], eh[:], AF.Ln, bias=1.0)
                pef = ps.tile([P, P], f32, tag="w", name="w")
                nc.tensor.matmul(pef[:], lhsT=W2_sb[:], rhs=hfm[:],
                                 start=True, stop=True, skip_group_check=True)
                eft = sb.tile([P, P], bf16, tag="eft", name="eft")
                nc.scalar.activation(eft[:], pef[:], AF.Identity,
                                     bias=b2_sb[:, :1])
                nc.sync.dma_start(ef_dram[:, t * P:(t + 1) * P], eft[:])

            # ---------------- layers ----------------
            for l in range(nl_build):
                nf_src = nf0 if l == 0 else ag_out[l - 1]

                # -- C1: q/k/v/qk tables (all nodes, replicated) --
                Wq_sb = wts.tile([128, D4], bf16, tag="Wq", name="Wq")
                nc.sync.dma_start(Wq_sb[:], Wq_f[l])
                Wk_sb = wts.tile([128, D4], bf16, tag="Wk", name="Wk")
                nc.sync.dma_start(Wk_sb[:], Wk[l])
                Wv_sb = wts.tile([128, D4], bf16, tag="Wv", name="Wv")
                nc.sync.dma_start(Wv_sb[:], Wv[l])
                We_sb = wts.tile([128, D4], bf16, tag="We", name="We")
                nc.sync.dma_start(We_sb[:], We[l])
                bq_bc = wts.tile([P, D4], bf16, tag="bq", name="bq")
                nc.sync.dma_start(bq_bc[:], bq_r[l].to_broadcast([P, D4]))
                bk_bc = wts.tile([P, D4], bf16, tag="bk", name="bk")
                nc.sync.dma_start(bk_bc[:], bk_r[l].to_broadcast([P, D4]))
                bv_bc = wts.tile([P, D4], bf16, tag="bv", name="bv")
                nc.sync.dma_start(bv_bc[:], bv_r[l].to_broadcast([P, D4]))
                be_sb = wts.tile([1, D4], bf16, tag="be", name="be")
                nc.sync.dma_start(be_sb[:], be_r[l])
                Wmu_sb = [wts.tile([128, D3], bf16, tag=f"Wmu{k}",
                                   name=f"Wmu{k}") for k in range(2)]
                for k in range(2):
                    nc.sync.dma_start(Wmu_sb[k][:],
                                      Wmu[l, k * 128:(k + 1) * 128, :])
                Wfold_sb = wts.tile([128, H * D3], bf16, tag="Wfold",
                                    name="Wfold")
                nc.sync.dma_start(Wfold_sb[:], Wfold[l])
                bmua_sb = wts.tile([1, H * D3], bf16, tag="bmua",
                                   name="bmua")
                nc.sync.dma_start(bmua_sb[:], bmua_r[l])
                Wmsg_sb = [wts.tile([128, C], bf16, tag=f"Wmsg{k}",
                                    name=f"Wmsg{k}") for k in range(3)]
                for k in range(3):
                    nc.sync.dma_start(Wmsg_sb[k][:],
                                      Wmsg[l, k * 128:(k + 1) * 128, :])
                bmsg_bc = wts.tile([P, D4], bf16, tag="bmsg", name="bmsg")
                nc.sync.dma_start(bmsg_bc[:], bmsg4_r[l].to_broadcast([P, D4]))
                Wc_sb = [wts.tile([128, 128], bf16, tag=f"Wc{k}",
                                  name=f"Wc{k}") for k in range(4)]
                for k in range(4):
                    nc.sync.dma_start(Wc_sb[k][:],
                                      Wc_f[l, k * 128:(k + 1) * 128, :])
                bc_sb = wts.tile([1, 128], bf16, tag="bc", name="bc")
                nc.sync.dma_start(bc_sb[:], bc_r[l])
                if not host["ln1_trivial"]:
                    g1_sb = wts.tile([P, H * D3], bf16, tag="g1", name="g1")
                    nc.sync.dma_start(g1_sb[:],
                                      g1_r[l].to_broadcast([P, H * D3]))
                    b1g_sb = wts.tile([P, H * D3], bf16, tag="b1g", name="b1g")
                    nc.sync.dma_start(b1g_sb[:],
                                      b1_r[l].to_broadcast([P, H * D3]))
                if not host["ln2b_trivial"]:
                    wdeg_sb = wts.tile([1, 128], bf16, tag="wdeg", name="wdeg")
                    nc.sync.dma_start(wdeg_sb[:], wdeg_r[l])
                    degrow_sb = wts.tile([1, RANGE], bf16, tag="degrow",
                                         name="degrow")
                    nc.sync.dma_start(degrow_sb[:], deg_row[:])

                for t in range(NT):
                    nftf = sb.tile([P, 128], f32, tag="nftf", name="nftf")
                    nc.sync.dma_start(nftf[:], nf_src[t * P:(t + 1) * P, :])
                    nft = sb.tile([P, 128], bf16, tag="nft", name="nft")
                    nc.vector.tensor_copy(nft[:], nftf[:])
                    pT = pst.tile([P, P], bf16, tag="t", name="t")
                    nc.tensor.transpose(pT[:], nft[:], ident[:])
                    nfT = sb.tile([P, P], bf16, tag="nfT", name="nfT")
                    nc.vector.tensor_copy(nfT[:], pT[:])
                    pq = ps.tile([P, D4], f32, tag="w", name="w")
                    nc.tensor.matmul(pq[:], lhsT=nfT[:], rhs=Wq_sb[:],
                                     start=True, stop=True,
                                     skip_group_check=True)
                    pk = ps.tile([P, D4], f32, tag="w", name="w")
                    nc.tensor.matmul(pk[:], lhsT=nfT[:], rhs=Wk_sb[:],
                                     start=True, stop=True,
                                     skip_group_check=True)
                    pv = ps.tile([P, D4], f32, tag="w", name="w")
                    nc.tensor.matmul(pv[:], lhsT=nfT[:], rhs=Wv_sb[:],
                                     start=True, stop=True,
                                     skip_group_check=True)
                    q_sb = sb2.tile([P, D4], bf16, tag="q_sb", name="q_sb")
                    nc.vector.tensor_tensor(out=q_sb[:], in0=pq[:],
                                            in1=bq_bc[:], op=OP.add)
                    k_sb = sb2.tile([P, D4], bf16, tag="k_sb", name="k_sb")
                    nc.vector.tensor_tensor(out=k_sb[:], in0=pk[:],
                                            in1=bk_bc[:], op=OP.add)
                    v_sb = sb2.tile([P, D4], bf16, tag="v_sb", name="v_sb")
                    nc.vector.tensor_tensor(out=v_sb[:], in0=pv[:],
                                            in1=bv_bc[:], op=OP.add)
                    qk_sb = sb2.tile([P, D4], bf16, tag="qk_sb", name="qk_sb")
                    nc.gpsimd.tensor_tensor(out=qk_sb[:], in0=q_sb[:],
                                            in1=k_sb[:], op=OP.mult)
                    nc.sync.dma_start(kv_tab[t * P:(t + 1) * P, :D4], k_sb[:])
                    nc.sync.dma_start(kv_tab[t * P:(t + 1) * P, D4:], v_sb[:])
                    nc.sync.dma_start(qkq_tab[t * P:(t + 1) * P, :D4],
                                      qk_sb[:])
                    nc.sync.dma_start(qkq_tab[t * P:(t + 1) * P, D4:],
                                      q_sb[:])

                # -- C2/C3: edge messages + scatter + node update --
                for c in range(CHUNKS):
                    qkq_ch = sbc.tile([P, 2 * D4], bf16, tag="qkq_ch",
                                      name="qkq_ch")
                    nc.gpsimd.indirect_dma_start(
                        out=qkq_ch[:], out_offset=None, in_=qkq_tab[:],
                        in_offset=bass.IndirectOffsetOnAxis(
                            ap=cidx_sb[:, c:c + 1], axis=0))
                    kv_ch = sbc.tile([P, 2 * D4], bf16, tag="kv_ch",
                                     name="kv_ch")
                    nc.gpsimd.indirect_dma_start(
                        out=kv_ch[:], out_offset=None, in_=kv_tab[:],
                        in_offset=bass.IndirectOffsetOnAxis(
                            ap=cidx_sb[:, c:c + 1], axis=0))
                    nf_old = sbc.tile([P, 128], f32, tag="nf_old",
                                      name="nf_old")
                    nc.gpsimd.indirect_dma_start(
                        out=nf_old[:], out_offset=None, in_=nf_src[:],
                        in_offset=bass.IndirectOffsetOnAxis(
                            ap=cidx_sb[:, c:c + 1], axis=0))
                    qk_ch = qkq_ch[:, :D4]
                    q_ch = qkq_ch[:, D4:]
                    v_ch = kv_ch[:, D4:]

                    # A-table: A = v@Wmu0 + (bmu + be@Wmu2), per head, for
                    # this chunk's 128 dst nodes.  Expanded to edges via oT
                    # directly into the m1 accumulation.
                    A_ch = sbc.tile([P, H * D3], bf16, tag="A_ch",
                                    name="A_ch")
                    pvf = pst.tile([P, D4], bf16, tag="t", name="t")
                    for h in range(H):
                        nc.tensor.transpose(
                            pvf[:, h * C:(h + 1) * C],
                            v_ch[:, h * C:(h + 1) * C], ident[:])
                    vfm_all = sb.tile([P, D4], bf16, tag="vfm_all",
                                      name="vfm_all")
                    nc.vector.tensor_copy(vfm_all[:], pvf[:])
                    for h in range(H):
                        pA = ps.tile([P, D3], f32, tag="w", name="w")
                        nc.tensor.matmul(pA[:],
                                         lhsT=vfm_all[:, h * C:(h + 1) * C],
                                         rhs=Wmu_sb[0][:],
                                         start=True, stop=False,
                                         skip_group_check=True)
                        nc.tensor.matmul(
                            pA[:], lhsT=ones_r[:, :P],
                            rhs=bmua_sb[:, h * D3:(h + 1) * D3],
                            start=False, stop=True, skip_group_check=True)
                        nc.vector.tensor_copy(
                            A_ch[:, h * D3:(h + 1) * D3], pA[:])

                    pagg = aggp.tile([P, D4], f32, tag="agg", name="agg")
                    for tt in range(TPC):
                        t = c * TPC + tt
                        # src-side gather: [k | v] rows by src index
                        kvs_g = sb.tile([P, 2 * D4], bf16, tag="kvs_g",
                                        name="kvs_g")
                        nc.gpsimd.indirect_dma_start(
                            out=kvs_g[:], out_offset=None, in_=kv_tab[:],
                            in_offset=bass.IndirectOffsetOnAxis(
                                ap=src_sb[:, t:t + 1], axis=0))
                        ks_g = kvs_g[:, :D4]
                        vs_g = kvs_g[:, D4:]
                        # host-precomputed one-hots
                        oh = sb.tile([P, P], bf16, tag="oh", name="oh")
                        nc.sync.dma_start(oh[:],
                                          oh_all[:, t * P:(t + 1) * P])
                        oT = sb.tile([P, P], bf16, tag="oT", name="oT")
                        nc.sync.dma_start(oT[:],
                                          oT_all[:, t * P:(t + 1) * P])
                        # dst-side expansions
                        pqkd = ps.tile([P, D4], f32, tag="w", name="w")
                        nc.tensor.matmul(pqkd[:], lhsT=oT[:], rhs=qk_ch,
                                         start=True, stop=True,
                                         skip_group_check=True)
                        pqd = ps.tile([P, D4], f32, tag="w", name="w")
                        nc.tensor.matmul(pqd[:], lhsT=oT[:], rhs=q_ch,
                                         start=True, stop=True,
                                         skip_group_check=True)
                        qd_sb = sb.tile([P, D4], bf16, tag="qd_sb",
                                        name="qd_sb")
                        nc.vector.tensor_copy(qd_sb[:], pqd[:])
                        # e (edge-major only; feature-major folded via Wfold)
                        eft = sb.tile([P, P], bf16, tag="eft2", name="eft2")
                        nc.sync.dma_start(eft[:],
                                          ef_dram[:, t * P:(t + 1) * P])
                        pe = ps.tile([P, D4], f32, tag="w", name="w")
                        nc.tensor.matmul(pe[:], lhsT=eft[:], rhs=We_sb[:],
                                         start=True, stop=False,
                                         skip_group_check=True)
                        nc.tensor.matmul(pe[:], lhsT=ones_r[:, :P],
                                         rhs=be_sb[:], start=False, stop=True,
                                         skip_group_check=True)
                        # v_s feature-major (batched transposes, one evac)
                        pvs = pst.tile([P, D4], bf16, tag="t", name="t")
                        for h in range(H):
                            nc.tensor.transpose(
                                pvs[:, h * C:(h + 1) * C],
                                vs_g[:, h * C:(h + 1) * C], ident[:])
                        vsfm = sb.tile([P, D4], bf16, tag="vsfm",
                                       name="vsfm")
                        nc.vector.tensor_copy(vsfm[:], pvs[:])
                        # alpha [P, H*3C], slot layout per head: [kd, ks, e]
                        alpha = sb2.tile([P, H * D3], bf16, tag="alpha",
                                         name="alpha")
                        a4 = alpha[:].rearrange("p (h s c) -> p h s c",
                                                h=H, s=3)
                        nc.vector.tensor_copy(
                            a4[:, :, 0, :],
                            pqkd[:].rearrange("p (h c) -> p h c", h=H))
                        nc.gpsimd.tensor_tensor(
                            out=a4[:, :, 1, :],
                            in0=qd_sb[:].rearrange("p (h c) -> p h c", h=H),
                            in1=ks_g.rearrange("p (h c) -> p h c", h=H),
                            op=OP.mult)
                        nc.vector.tensor_tensor(
                            out=a4[:, :, 2, :],
                            in0=qd_sb[:].rearrange("p (h c) -> p h c", h=H),
                            in1=pe[:].rearrange("p (h c) -> p h c", h=H),
                            op=OP.mult)
                        # LN1 + sigmoid -> gate
                        su4 = sb.tile([P, H], f32, tag="su4", name="su4")
                        nc.vector.reduce_sum(
                            su4[:], alpha[:].rearrange("p (h f) -> p h f",
                                                       h=H), axis=AX.X)
                        nmu4 = sb.tile([P, H], f32, tag="nmu4", name="nmu4")
                        nc.vector.tensor_scalar_mul(nmu4[:], su4[:],
                                                    -1.0 / D3)
                        vs4 = sb.tile([P, H], f32, tag="vs4", name="vs4")
                        scr = sb.tile([P, D3], bf16, tag="scr", name="scr")
                        for h in range(H):
                            nc.scalar.activation(
                                scr[:], alpha[:, h * D3:(h + 1) * D3],
                                AF.Square, bias=nmu4[:, h:h + 1],
                                accum_out=vs4[:, h:h + 1])
                        v4 = sb.tile([P, H], f32, tag="v4", name="v4")
                        nc.vector.tensor_scalar(out=v4[:], in0=vs4[:],
                                                scalar1=1.0 / D3,
                                                scalar2=EPS, op0=OP.mult,
                                                op1=OP.add)
                        rstd4 = rsqrt(v4[:], [P, H], nc.vector)
                        nmr4 = sb.tile([P, H], f32, tag="nmr4", name="nmr4")
                        nc.vector.tensor_tensor(out=nmr4[:], in0=nmu4[:],
                                                in1=rstd4, op=OP.mult)
                        gate = sb2.tile([P, H * D3], bf16, tag="gate",
                                        name="gate")
                        if host["ln1_trivial"]:
                            for h in range(H):
                                nc.scalar.activation(
                                    gate[:, h * D3:(h + 1) * D3],
                                    alpha[:, h * D3:(h + 1) * D3],
                                    AF.Sigmoid, scale=rstd4[:, h:h + 1],
                                    bias=nmr4[:, h:h + 1])
                        else:
                            xn = sb2.tile([P, H * D3], bf16, tag="xn",
                                          name="xn")
                            for h in range(H):
                                nc.scalar.activation(
                                    xn[:, h * D3:(h + 1) * D3],
                                    alpha[:, h * D3:(h + 1) * D3],
                                    AF.Identity, scale=rstd4[:, h:h + 1],
                                    bias=nmr4[:, h:h + 1])
                            nc.vector.tensor_tensor(out=xn[:], in0=xn[:],
                                                    in1=g1_sb[:], op=OP.mult)
                            nc.vector.tensor_tensor(out=xn[:], in0=xn[:],
                                                    in1=b1g_sb[:], op=OP.add)
                            nc.scalar.activation(gate[:], xn[:], AF.Sigmoid)
                        # m1 = [vd|vs|e] @ Wmu + bmu (A-table + Wfold carry
                        # the vd and e parts); m1g = m1 * gate
                        m1g = sb2.tile([P, H * D3], bf16, tag="m1g",
                                       name="m1g")
                        for h in range(H):
                            pm1 = ps.tile([P, D3], f32, tag="w", name="w")
                            nc.tensor.matmul(
                                pm1[:], lhsT=oT[:],
                                rhs=A_ch[:, h * D3:(h + 1) * D3],
                                start=True, stop=False,
                                skip_group_check=True)
                            nc.tensor.matmul(pm1[:],
                                             lhsT=vsfm[:, h * C:(h + 1) * C],
                                             rhs=Wmu_sb[1][:],
                                             start=False, stop=False,
                                             skip_group_check=True)
                            nc.tensor.matmul(
                                pm1[:], lhsT=eft[:],
                                rhs=Wfold_sb[:, h * D3:(h + 1) * D3],
                                start=False, stop=True,
                                skip_group_check=True)
                            nc.vector.tensor_tensor(
                                out=m1g[:, h * D3:(h + 1) * D3], in0=pm1[:],
                                in1=gate[:, h * D3:(h + 1) * D3], op=OP.mult)
                        # m2 = m1g @ Wmsg (+bmsg in evac); col 128 is the
                        # free row-sum for LN2's mean
                        m_fin = sb2.tile([P, D4], bf16, tag="m_fin",
                                         name="m_fin")
                        m2sb = sb2.tile([P, D4], bf16, tag="m2sb",
                                        name="m2sb")
                        pm2 = ps.tile([P, D4], f32, tag="w", name="w")
                        # zero-fill + set has_written on the whole bank so
                        # the 12 per-head matmuls below can accumulate in
                        # any order (their slices are disjoint across heads)
                        nc.tensor.matmul(pm2[:], lhsT=zero_r[:, :P],
                                         rhs=ones_r[:], start=True,
                                         stop=False, skip_group_check=True)
                        for h in range(H):
                            pmT = pst.tile([P, D3], bf16, tag="t",
                                           name="t")
                            for k in range(3):
                                nc.tensor.transpose(
                                    pmT[:, k * 128:(k + 1) * 128],
                                    m1g[:, h * D3 + k * 128:
                                        h * D3 + (k + 1) * 128], ident[:])
                            mT = sb.tile([P, D3], bf16, tag="mT", name="mT")
                            if h % 2 == 0:
                                nc.vector.tensor_copy(mT[:], pmT[:])
                            else:
                                nc.scalar.copy(mT[:], pmT[:])
                            for k in range(3):
                                nc.tensor.matmul(
                                    pm2[:, h * C:(h + 1) * C],
                                    lhsT=mT[:, k * 128:(k + 1) * 128],
                                    rhs=Wmsg_sb[k][:],
                                    start=False,
                                    stop=(h == H - 1 and k == 2),
                                    skip_group_check=True)
                        nc.vector.tensor_tensor(
                            out=m2sb[:], in0=pm2[:], in1=bmsg_bc[:],
                            op=OP.add)
                        s24 = sb.tile([P, H], f32, tag="s24", name="s24")
                        nc.vector.reduce_sum(
                            s24[:], m2sb[:].rearrange("p (h f) -> p h f",
                                                      h=H), axis=AX.X)
                        nmu24 = sb.tile([P, H], f32, tag="nmu24",
                                        name="nmu24")
                        nc.vector.tensor_scalar_mul(nmu24[:], s24[:],
                                                    -1.0 / C)
                        vs24 = sb.tile([P, H], f32, tag="vs24", name="vs24")
                        scr2 = sb.tile([P, C], bf16, tag="scr2", name="scr2")
                        for h in range(H):
                            nc.scalar.activation(
                                scr2[:], m2sb[:, h * C:(h + 1) * C],
                                AF.Square, bias=nmu24[:, h:h + 1],
                                accum_out=vs24[:, h:h + 1])
                        v24 = sb.tile([P, H], f32, tag="v24", name="v24")
                        nc.vector.tensor_scalar(out=v24[:], in0=vs24[:],
                                                scalar1=1.0 / C, scalar2=EPS,
                                                op0=OP.mult, op1=OP.add)
                        rstd24 = rsqrt(v24[:], [P, H], nc.vector)
                        nmr24 = sb.tile([P, H], f32, tag="nmr24",
                                        name="nmr24")
                        nc.vector.tensor_tensor(out=nmr24[:], in0=nmu24[:],
                                                in1=rstd24, op=OP.mult)
                        for h in range(H):
                            nc.gpsimd.tensor_scalar(
                                out=m_fin[:, h * C:(h + 1) * C],
                                in0=m2sb[:, h * C:(h + 1) * C],
                                scalar1=rstd24[:, h:h + 1],
                                scalar2=nmr24[:, h:h + 1],
                                op0=OP.mult, op1=OP.add)
                        # scatter-add into chunk aggregate
                        nc.tensor.matmul(pagg[:], lhsT=oh[:], rhs=m_fin[:],
                                         start=(tt == 0), stop=(tt == TPC - 1),
                                         skip_group_check=True)

                    # -- node update for chunk c --
                    agg_sb = sb2.tile([P, D4], bf16, tag="agg_sb",
                                      name="agg_sb")
                    nc.vector.tensor_copy(agg_sb[:], pagg[:])
                    pout = ps.tile([P, 128], f32, tag="w", name="w")
                    for k in range(4):
                        paT = pst.tile([P, P], bf16, tag="t", name="t")
                        nc.tensor.transpose(
                            paT[:], agg_sb[:, k * 128:(k + 1) * 128],
                            ident[:])
                        aT = sb.tile([P, P], bf16, tag="aT", name="aT")
                        nc.vector.tensor_copy(aT[:], paT[:])
                        nc.tensor.matmul(pout[:], lhsT=aT[:], rhs=Wc_sb[k][:],
                                         start=(k == 0), stop=False,
                                         skip_group_check=True)
                    last = host["ln2b_trivial"]
                    nc.tensor.matmul(pout[:], lhsT=ones_r[:, :P],
                                     rhs=bc_sb[:], start=False, stop=last,
                                     skip_group_check=True)
                    if not host["ln2b_trivial"]:
                        nc.tensor.matmul(
                            pout[:],
                            lhsT=degrow_sb[:, c * P:(c + 1) * P],
                            rhs=wdeg_sb[:], start=False, stop=True,
                            skip_group_check=True)
                    nfn = sb.tile([P, 128], f32, tag="nfn", name="nfn")
                    nc.vector.tensor_tensor(out=nfn[:], in0=pout[:],
                                            in1=nf_old[:], op=OP.add)
                    sgx = sb.tile([P, 128], f32, tag="sgx", name="sgx")
                    nc.scalar.activation(sgx[:], nfn[:], AF.Sigmoid)
                    nfo = sb.tile([P, 128], f32, tag="nfo", name="nfo")
                    nc.gpsimd.tensor_tensor(out=nfo[:], in0=nfn[:],
                                            in1=sgx[:], op=OP.mult)
                    nc.sync.dma_start(ag_in[l][c * P:(c + 1) * P, :], nfo[:])

                # -- C4: allgather nf --
                nc.gpsimd.collective_compute(
                    "AllGather", OP.bypass,
                    ins=[ag_in[l].opt()],
                    outs=[ag_out[l].opt()],
                    replica_groups=[list(range(NCORES))],
                )

            # ---------------- pooling + readout ----------------
            gidA_sb = cpool.tile([P, CHUNKS], f32, name="t120")
            nc.sync.dma_start(gidA_sb[:], gidA_T[:])
            gidB_sb = cpool.tile([P, CHUNKS], f32, name="t121")
            nc.sync.dma_start(gidB_sb[:], gidB_T[:])
            nf_fin = ag_in[nl_build - 1]
            ppA = aggp.tile([P, 129], f32, tag="agg", name="agg")
            ppB = aggp.tile([P, 129], f32, tag="agg", name="agg")
            for c in range(CHUNKS):
                rhs = sb.tile([P, 129], f32, tag="prhs", name="prhs")
                nc.sync.dma_start(rhs[:, :128],
                                  nf_fin[c * P:(c + 1) * P, :])
                nc.gpsimd.memset(rhs[:, 128:129], 1.0)
                ohA = sb.tile([P, P], f32, tag="ohA", name="ohA")
                nc.vector.tensor_tensor(
                    out=ohA[:], in0=gidA_sb[:, c:c + 1].to_broadcast([P, P]),
                    in1=iota_f[:], op=OP.is_equal)
                ohB = sb.tile([P, P], f32, tag="ohB", name="ohB")
                nc.vector.tensor_tensor(
                    out=ohB[:], in0=gidB_sb[:, c:c + 1].to_broadcast([P, P]),
                    in1=iota_f[:], op=OP.is_equal)
                nc.tensor.matmul(ppA[:], lhsT=ohA[:], rhs=rhs[:],
                                 start=(c == 0), stop=(c == CHUNKS - 1),
                                 skip_group_check=True)
                nc.tensor.matmul(ppB[:], lhsT=ohB[:], rhs=rhs[:],
                                 start=(c == 0), stop=(c == CHUNKS - 1),
                                 skip_group_check=True)
            pA_sb = sb.tile([P, 129], f32, tag="pA_sb", name="pA_sb")
            nc.vector.tensor_copy(pA_sb[:], ppA[:])
            pB_sb = sb.tile([P, 129], f32, tag="pB_sb", name="pB_sb")
            nc.vector.tensor_copy(pB_sb[:], ppB[:])
            nc.sync.dma_start(ar_in[0:P, :], pA_sb[:])
            nc.sync.dma_start(ar_in[P:2 * P, :], pB_sb[:])
            nc.gpsimd.collective_compute(
                "AllReduce", OP.add,
                ins=[ar_in.opt()],
                outs=[ar_out.opt()],
                replica_groups=[list(range(NCORES))],
            )
            fcW_sb = cpool.tile([128, 128], f32, name="t129")
            nc.sync.dma_start(fcW_sb[:], fc_W[:])
            fcb_sb = cpool.tile([1, 128], f32, name="t130")
            nc.sync.dma_start(fcb_sb[:], fc_b_r[:])
            outW_sb = cpool.tile([128, 1], f32, name="t131")
            nc.sync.dma_start(outW_sb[:], out_W[:])
            for half in range(2):
                pool_t = sb.tile([P, 129], f32, tag="pool_t", name="pool_t")
                nc.sync.dma_start(pool_t[:],
                                  ar_out[half * P:(half + 1) * P, :])
                cnt = sb.tile([P, 1], f32, tag="cnt", name="cnt")
                nc.vector.tensor_scalar_max(cnt[:], pool_t[:, 128:129], 1.0)
                rc = sb.tile([P, 1], f32, tag="rc", name="rc")
                nc.vector.reciprocal(rc[:], cnt[:])
                pooled = sb.tile([P, 128], f32, tag="pooled", name="pooled")
                nc.vector.tensor_scalar_mul(pooled[:], pool_t[:, :128],
                                            rc[:, :1])
                ppT = pst.tile([P, P], f32, tag="t", name="t")
                nc.tensor.transpose(ppT[:], pooled[:], identf[:])
                poolT = sb.tile([P, P], f32, tag="poolT", name="poolT")
                nc.vector.tensor_copy(poolT[:], ppT[:])
                pf = ps.tile([P, 128], f32, tag="w", name="w")
                nc.tensor.matmul(pf[:], lhsT=poolT[:], rhs=fcW_sb[:],
                                 start=True, stop=False,
                                 skip_group_check=True)
                nc.tensor.matmul(pf[:], lhsT=onesf[:], rhs=fcb_sb[:],
                                 start=False, stop=True,
                                 skip_group_check=True)
                sgf = sb.tile([P, 128], f32, tag="sgf", name="sgf")
                nc.scalar.activation(sgf[:], pf[:], AF.Sigmoid)
                feats = sb.tile([P, 128], f32, tag="feats", name="feats")
                nc.vector.tensor_tensor(out=feats[:], in0=pf[:],
                                        in1=sgf[:], op=OP.mult)
                pfT = pst.tile([P, P], f32, tag="t", name="t")
                nc.tensor.transpose(pfT[:], feats[:], identf[:])
                featT = sb.tile([P, P], f32, tag="featT", name="featT")
                nc.vector.tensor_copy(featT[:], pfT[:])
                po = ps.tile([P, 1], f32, tag="w", name="w")
                nc.tensor.matmul(po[:], lhsT=featT[:], rhs=outW_sb[:],
                                 start=True, stop=True,
                                 skip_group_check=True)
                yt = sb.tile([P, 1], f32, tag="yt", name="yt")
                nc.vector.tensor_scalar_add(yt[:], po[:],
                                            host["out_b_val"])
                nc.sync.dma_start(y[half * P:(half + 1) * P, :], yt[:])

    nc.finalize()
    return nc


def _in_maps(host):
    shared = {k: host[k] for k in
              ["x_fm", "emb_W", "emb_b_r", "rbf_W1", "rbf_W2", "b1_col",
               "b2_col", "centers_r", "Wq_f", "bq_r", "Wk", "bk_r", "Wv",
               "bv_r", "We", "be_r", "Wmu", "Wfold", "bmua_r", "Wmsg",
               "bmsg4_r", "Wc_f", "bc_r", "fc_W", "fc_b_r", "out_W"]}
    if not host["ln1_trivial"]:
        shared["g1_r"] = host["g1_r"]
        shared["b1_r"] = host["b1_r"]
    maps = []
    for i in range(NCORES):
        pc = host["percore"][i]
        m = dict(shared)
        m["src_T"] = pc["src_T"]
        m["ea_s"] = pc["ea_s"]
        m["oh_all"] = pc["oh_all"]
        m["oT_all"] = pc["oT_all"]
        m["chunk_idx_T"] = pc["chunk_idx_T"]
        m["gidA_T"] = pc["gidA_T"]
        m["gidB_T"] = pc["gidB_T"]
        if not host["ln2b_trivial"]:
            m["wdeg_r"] = host["wdeg_r"]
            m["deg_row"] = pc["deg_row"]
        maps.append(m)
    return maps


def kernel(_trace=False, **inputs):
    from concourse import bass_utils
    host = _prep(inputs)
    nc = _build(host, nl_build=_NL_BUILD)
    res = bass_utils.run_bass_kernel_spmd(
        nc, _in_maps(host), core_ids=list(range(NCORES)), trace=_trace)
    y = np.asarray(res.results[0]["y"])[:, 0].astype(np.float32)
    if _trace:
        kernel.last_result = res
    return y
